# revision 1
# baseline (speedup 1.0000x reference)
"""Trainium2 Bass kernel for nn_DAGNLinkPrediction (GNN message passing).

Self-contained: host-side integer preprocessing (sharding/permutation) + bass/tile
kernel + SPMD launch across 8 NeuronCores via run_bass_kernel_spmd.

Sharding: edges partitioned by src range (6272 nodes/core). Per core, edges are
sorted by (dst>=SPLIT, src-block, src) and padded into 128-edge chunks confined
to 128-node blocks. Per power iteration each core gathers Z[dst] rows (256B bf16)
with gpsimd.dma_gather, computes messages, segment-sums by src via PE matmuls
with 0/1 one-hot matrices, and AllGathers the updated bf16 node table.
"""
import numpy as np
import ml_dtypes

from concourse import bass, bacc, tile, bass_utils, mybir

BF = ml_dtypes.bfloat16
F32 = mybir.dt.float32
BF16 = mybir.dt.bfloat16
I16 = mybir.dt.int16

CORES = 8
N_ENT = 50000
N_REL = 500
HEADS = 2
DIM = 64
HD = HEADS * DIM                 # 128
M_PER_CORE = 6272                # 49*128
NBLK = M_PER_CORE // 128         # 49
TAB_ROWS = CORES * M_PER_CORE    # 50176
SPLIT = TAB_ROWS // 2            # 25088
NPIECE = 7                       # pieces per pass; NBLK = 7*7
BPP = NBLK // NPIECE             # blocks per piece = 7
ALPHA = 0.15
LN_EPS = 1e-5
L_LAYERS = 2
POW_ITER = 3
LRELU = 0.01

AF = mybir.ActivationFunctionType
OP = mybir.AluOpType


# ----------------------------------------------------------------------------
# host-side preprocessing (integer/layout only)
# ----------------------------------------------------------------------------

def _wrap_idxs(idx):
    n = idx.shape[0]
    base = idx.reshape(n // 16, 16).T.astype(np.int16)
    return np.ascontiguousarray(np.tile(base, (8, 1)))


def _build_plan(edge_index, edge_type):
    src = edge_index[0].astype(np.int64)
    dst = edge_index[1].astype(np.int64)
    et = edge_type.astype(np.int64)

    core_of = src // M_PER_CORE
    hi = (dst >= SPLIT).astype(np.int64)
    blk = (src % M_PER_CORE) // 128
    key = (core_of * 2 + hi) * NBLK + blk
    order = np.argsort(key, kind="stable")
    key_sorted = key[order]
    bounds = np.searchsorted(key_sorted, np.arange(CORES * 2 * NBLK + 1))

    counts = (bounds[1:] - bounds[:-1]).reshape(CORES, 2, NBLK)
    CPB = max(1, int(np.ceil(counts.max() / 128)))
    C_PASS = NBLK * CPB
    NIDX = C_PASS * 128

    plans = []
    for c in range(CORES):
        pl = {}
        pl["node_lo"] = c * M_PER_CORE
        pl["n_valid"] = max(0, min(N_ENT - c * M_PER_CORE, M_PER_CORE))
        gidx, offs, typs = [], [], []
        for p in range(2):
            eids = np.full(NIDX, -1, np.int64)
            for b in range(NBLK):
                k = (c * 2 + p) * NBLK + b
                lst = order[bounds[k]:bounds[k + 1]]
                eids[b * CPB * 128: b * CPB * 128 + len(lst)] = lst
            pad = eids < 0
            e_safe = np.where(pad, 0, eids)
            didx = dst[e_safe] - p * SPLIT
            didx[pad] = 0
            off = (src[e_safe] % 128).astype(np.int64)
            off[pad] = -1
            t = et[e_safe].copy()
            t[pad] = 0
            gidx.append(_wrap_idxs(didx.astype(np.int16)))
            offs.append(off.reshape(C_PASS, 128).T.astype(np.float32))  # [128, C_PASS]
            typs.append(t)
        pl["gidx"] = gidx
        pl["tidx"] = _wrap_idxs(np.concatenate(typs).astype(np.int16))
        shl = []
        for p in range(2):
            off_flat = offs[p].T.reshape(-1)  # [NIDX] slot-order src offsets (-1 pads)
            # rebuild local src ids from block index + offset
            blk_of_slot = np.repeat(np.arange(NBLK), CPB * 128)
            sl = blk_of_slot * 128 + np.maximum(off_flat, 0).astype(np.int64)
            sl[off_flat < 0] = 0
            shl.append(sl)
        pl["shidx"] = _wrap_idxs(np.concatenate(shl).astype(np.int16))
        pl["offs"] = np.concatenate(offs, axis=1).astype(BF)  # [128, 2*C_PASS]
        plans.append(pl)

    meta = dict(CPB=CPB, C_PASS=C_PASS, NIDX=NIDX,
                PIECE_CHUNKS=BPP * CPB, NIDX_PIECE=BPP * CPB * 128)
    return plans, meta


# ----------------------------------------------------------------------------
# device kernel
# ----------------------------------------------------------------------------

def _build_nc(meta, debug=False, n_layers=L_LAYERS, n_iters=POW_ITER, do_ag=True, do_node_ag=True):
    CPB = meta["CPB"]
    C_PASS = meta["C_PASS"]
    PC = meta["PIECE_CHUNKS"]          # chunks per piece
    NP_IDX = meta["NIDX_PIECE"]        # idxs per piece
    IW = meta["NIDX"] // 16            # idx cols per pass

    nc = bacc.Bacc("TRN2", target_bir_lowering=False, debug=False,
                   num_devices=CORES)

    # ---- external inputs ----
    ent_in = nc.dram_tensor("ent_slice", [M_PER_CORE, DIM], F32, kind="ExternalInput")
    rel_in = nc.dram_tensor("rel_emb", [512, DIM], F32, kind="ExternalInput")
    lng_in = nc.dram_tensor("ln_g", [128, L_LAYERS, DIM], F32, kind="ExternalInput")
    lnb_in = nc.dram_tensor("ln_b", [128, L_LAYERS, DIM], F32, kind="ExternalInput")
    W_in = nc.dram_tensor("W_htr", [3, L_LAYERS, DIM, HD], F32, kind="ExternalInput")
    att_in = nc.dram_tensor("att_rep", [3, L_LAYERS, 128, HD], F32, kind="ExternalInput")
    Wo_in = nc.dram_tensor("W_o", [L_LAYERS, HD, DIM], F32, kind="ExternalInput")
    gidx_in = nc.dram_tensor("gidx", [128, 2, IW], I16, kind="ExternalInput")
    tidx_in = nc.dram_tensor("tidx", [128, 2 * IW], I16, kind="ExternalInput")
    shidx_in = nc.dram_tensor("shidx", [128, 2 * IW], I16, kind="ExternalInput")
    offs_in = nc.dram_tensor("offs", [128, 2 * C_PASS], BF16, kind="ExternalInput")
    iota_in = nc.dram_tensor("iota", [128, 128], BF16, kind="ExternalInput")
    idf_in = nc.dram_tensor("ident_f", [128, 128], F32, kind="ExternalInput")

    out_ext = nc.dram_tensor("out_slice", [M_PER_CORE, DIM], F32, kind="ExternalOutput")
    if debug:
        dbg_at = nc.dram_tensor("dbg_at", [128, 2 * C_PASS, 2], BF16, kind="ExternalOutput")
        dbg_z0 = nc.dram_tensor("dbg_z0", [M_PER_CORE, HD], F32, kind="ExternalOutput")
        dbg_h = nc.dram_tensor("dbg_h", [M_PER_CORE, DIM], F32, kind="ExternalOutput")

    with tile.TileContext(nc) as tc:
        with tc.tile_pool(name="dram", bufs=1, space="DRAM") as dram, \
             tc.tile_pool(name="persist", bufs=1) as pp:
            table = dram.tile([TAB_ROWS, 128], BF16, tag="table")
            tab_in = dram.tile([M_PER_CORE, 128], BF16, tag="tab_in")
            srtab = dram.tile([512, 128], BF16, tag="srtab")
            shtab = dram.tile([M_PER_CORE, 128], BF16, tag="shtab")
            ohcache = dram.tile([128, 2 * C_PASS, 128], BF16, tag="ohcache")

            ent = pp.tile([128, NBLK, DIM], F32, tag="ent")
            h_t = pp.tile([128, NBLK, DIM], F32, tag="h")
            recip = pp.tile([128, NBLK, 2], F32, tag="recip")
            zt = pp.tile([128, NBLK, 2], F32, tag="zt")
            At = pp.tile([128, 2 * C_PASS, 2], BF16, tag="At")
            SRx = pp.tile([128, 2 * C_PASS, 8], BF16, tag="SRx")
            Z = pp.tile([128, NBLK, HD], F32, tag="Z")
            gidx = pp.tile([128, 2, IW], I16, tag="gidx")
            tidx = pp.tile([128, 2 * IW], I16, tag="tidx")
            shidx = pp.tile([128, 2 * IW], I16, tag="shidx")
            offs = pp.tile([128, 2 * C_PASS], BF16, tag="offs")
            iota = pp.tile([128, 128], BF16, tag="iota")
            idf = pp.tile([128, 128], F32, tag="idf")
            lng = pp.tile([128, L_LAYERS, DIM], F32, tag="lng")
            lnb = pp.tile([128, L_LAYERS, DIM], F32, tag="lnb")
            Wht = pp.tile([64, 3 * L_LAYERS, HD], F32, tag="Wht")
            attr = pp.tile([128, 3 * L_LAYERS, HD], F32, tag="attr")
            Wo = pp.tile([HD, L_LAYERS, DIM], F32, tag="Wo")

            # ---- load inputs ----
            nc.sync.dma_start(ent[:, :, :], ent_in.ap().rearrange("(b p) f -> p b f", p=128))
            nc.sync.dma_start(gidx[:, :, :], gidx_in.ap())
            nc.sync.dma_start(tidx[:, :], tidx_in.ap())
            nc.sync.dma_start(shidx[:, :], shidx_in.ap())
            nc.sync.dma_start(offs[:, :], offs_in.ap())
            nc.sync.dma_start(iota[:, :], iota_in.ap())
            nc.sync.dma_start(idf[:, :], idf_in.ap())
            nc.sync.dma_start(lng[:, :, :], lng_in.ap())
            nc.sync.dma_start(lnb[:, :, :], lnb_in.ap())
            nc.sync.dma_start(Wht[:, :, :], W_in.ap().rearrange("r l k m -> k (r l) m"))
            nc.sync.dma_start(attr[:, :, :], att_in.ap().rearrange("r l p m -> p (r l) m"))
            nc.sync.dma_start(Wo[:, :, :], Wo_in.ap().rearrange("l k m -> k l m"))

            def node_scores_block(pool, psum, lhsT, Wslice, att_ap, out_ap):
                """tanh(x@W) . att summed over d -> out_ap [128,2] (f32)."""
                pt = psum.tile([128, HD], F32, tag="ns_ps")
                nc.tensor.matmul(pt[:, :], lhsT, Wslice, start=True, stop=True)
                tt = pool.tile([128, HD], F32, tag="ns_tt")
                nc.scalar.activation(tt[:, :], pt[:, :], AF.Tanh)
                tm = pool.tile([128, HD], F32, tag="ns_tm")
                nc.vector.tensor_tensor(tm[:, :], tt[:, :], att_ap, OP.mult)
                nc.vector.tensor_reduce(out_ap, tm.rearrange("p (h d) -> p h d", h=2),
                                        mybir.AxisListType.X, OP.add)

            for l in range(n_layers):
                # ================= node phase =================
                with tc.tile_pool(name="nodep", bufs=2) as np_pool, \
                     tc.tile_pool(name="nodebig", bufs=1) as np_big, \
                     tc.tile_pool(name="nodeps", bufs=2, space="PSUM") as np_psum:
                    # layernorm
                    x = ent
                    mu = np_pool.tile([128, NBLK], F32, tag="mu")
                    nc.vector.tensor_reduce(mu[:, :], x[:, :, :], mybir.AxisListType.X, OP.add)
                    nc.vector.tensor_scalar(mu[:, :], mu[:, :], 1.0 / DIM, None, OP.mult)
                    xc = np_big.tile([128, NBLK, DIM], F32, tag="xc")
                    nc.vector.tensor_tensor(
                        xc[:, :, :], x[:, :, :],
                        mu.unsqueeze(2).broadcast_to([128, NBLK, DIM]), OP.subtract)
                    sq = np_big.tile([128, NBLK, DIM], F32, tag="sq")
                    nc.vector.tensor_tensor(sq[:, :, :], xc[:, :, :], xc[:, :, :], OP.mult)
                    var = np_pool.tile([128, NBLK], F32, tag="var")
                    nc.vector.tensor_reduce(var[:, :], sq[:, :, :], mybir.AxisListType.X, OP.add)
                    nc.vector.tensor_scalar(var[:, :], var[:, :], 1.0 / DIM, LN_EPS, OP.mult, OP.add)
                    std = np_pool.tile([128, NBLK], F32, tag="std")
                    nc.scalar.activation(std[:, :], var[:, :], AF.Sqrt)
                    rstd = np_pool.tile([128, NBLK], F32, tag="rstd")
                    nc.vector.reciprocal(rstd[:, :], std[:, :])
                    nc.vector.tensor_tensor(
                        h_t[:, :, :], xc[:, :, :],
                        rstd.unsqueeze(2).broadcast_to([128, NBLK, DIM]), OP.mult)
                    nc.vector.tensor_tensor(
                        h_t[:, :, :], h_t[:, :, :],
                        lng[:, l, :].unsqueeze(1).broadcast_to([128, NBLK, DIM]),
                        OP.mult)
                    nc.vector.tensor_tensor(
                        h_t[:, :, :], h_t[:, :, :],
                        lnb[:, l, :].unsqueeze(1).broadcast_to([128, NBLK, DIM]),
                        OP.add)
                    if debug and l == 0:
                        nc.sync.dma_start(dbg_h.ap().rearrange("(b p) f -> p b f", p=128),
                                          h_t[:, :, :])

                    # transpose h -> ht [64, b, 128]
                    ht = np_big.tile([64, NBLK, 128], F32, tag="ht")
                    for b in range(NBLK):
                        ps = np_psum.tile([64, 128], F32, tag="trh")
                        nc.tensor.transpose(ps[:, :], h_t[:, b, :], idf[:, :])
                        nc.scalar.activation(ht[:, b, :], ps[:, :], AF.Copy)

                    # s_h, s_t  [128, NBLK, 2] f32
                    s_f0 = np_pool.tile([128, NBLK, 2], F32, tag="s_f0")
                    s_f1 = np_pool.tile([128, NBLK, 2], F32, tag="s_f1")
                    s_f = [s_f0, s_f1]
                    for r in range(2):
                        for b in range(NBLK):
                            node_scores_block(np_pool, np_psum, ht[:, b, :],
                                              Wht[:, r * L_LAYERS + l, :],
                                              attr[:, r * L_LAYERS + l, :],
                                              s_f[r][:, b, :])
                    # sh table rows: [bf16(s_h)(2), bf16(residual)(2), 0...]
                    shsb = np_big.tile([128, NBLK, 128], BF16, tag="shsb")
                    shm_f = np_pool.tile([128, NBLK, 2], F32, tag="shm_f")
                    nc.vector.memset(shsb[:, :, 4:128], 0.0)
                    nc.vector.tensor_copy(shsb[:, :, 0:2], s_f[0][:, :, :])
                    nc.vector.tensor_copy(shm_f[:, :, :], shsb[:, :, 0:2])
                    nc.vector.tensor_tensor(shsb[:, :, 2:4], s_f[0][:, :, :], shm_f[:, :, :],
                                            OP.subtract)
                    nc.sync.dma_start(shtab.rearrange("(b p) c -> p b c", p=128),
                                      shsb[:, :, :])

                    # gather table slice: [h | s_t | s_t_res | 1 | 0...]
                    tabsb = np_big.tile([128, NBLK, 128], BF16, tag="tabsb")
                    nc.vector.tensor_copy(tabsb[:, :, 0:64], h_t[:, :, :])
                    nc.vector.tensor_copy(tabsb[:, :, 64:66], s_f[1][:, :, :])
                    stm_f = np_pool.tile([128, NBLK, 2], F32, tag="stm_f")
                    nc.vector.tensor_copy(stm_f[:, :, :], tabsb[:, :, 64:66])
                    nc.vector.tensor_tensor(tabsb[:, :, 66:68], s_f[1][:, :, :], stm_f[:, :, :],
                                            OP.subtract)
                    nc.vector.memset(tabsb[:, :, 68:69], 1.0)
                    nc.vector.memset(tabsb[:, :, 69:128], 0.0)
                    nc.sync.dma_start(tab_in.rearrange("(b p) c -> p b c", p=128),
                                      tabsb[:, :, :])
                    if do_node_ag:
                        nc.gpsimd.collective_compute(
                            "AllGather", OP.bypass,
                            replica_groups=[list(range(CORES))],
                            ins=[tab_in.opt()], outs=[table.opt()])
                    else:
                        nc.sync.dma_start(table[0:M_PER_CORE, :],
                                          tab_in[:, :])

                    # s_r table (once, both layers)
                    if l == 0:
                        relsb = np_pool.tile([128, 4, DIM], F32, tag="relsb")
                        nc.sync.dma_start(relsb[:, :, :],
                                          rel_in.ap().rearrange("(b p) f -> p b f", p=128))
                        relt = np_pool.tile([64, 4, 128], F32, tag="relt")
                        for b in range(4):
                            ps = np_psum.tile([64, 128], F32, tag="trh")
                            nc.tensor.transpose(ps[:, :], relsb[:, b, :], idf[:, :])
                            nc.scalar.activation(relt[:, b, :], ps[:, :], AF.Copy)
                        srsb = np_pool.tile([128, 4, 128], BF16, tag="srsb")
                        nc.vector.memset(srsb[:, :, :], 0.0)
                        sr_f = np_pool.tile([128, 4, 2 * L_LAYERS], F32, tag="sr_f")
                        for ll in range(L_LAYERS):
                            for b in range(4):
                                node_scores_block(np_pool, np_psum, relt[:, b, :],
                                                  Wht[:, 2 * L_LAYERS + ll, :],
                                                  attr[:, 2 * L_LAYERS + ll, :],
                                                  sr_f[:, b, 2 * ll:2 * ll + 2])
                        nc.vector.tensor_copy(srsb[:, :, 0:4], sr_f[:, :, :])
                        srm_f = np_pool.tile([128, 4, 4], F32, tag="srm_f")
                        nc.vector.tensor_copy(srm_f[:, :, :], srsb[:, :, 0:4])
                        nc.vector.tensor_tensor(srsb[:, :, 4:8], sr_f[:, :, :], srm_f[:, :, :],
                                                OP.subtract)
                        nc.sync.dma_start(srtab.rearrange("(b p) c -> p b c", p=128),
                                          srsb[:, :, :])

                # ================= edge phase =================
                for it in range(n_iters):
                    with tc.tile_pool(name="edgep", bufs=3) as ep, \
                         tc.tile_pool(name="edgeps", bufs=2) as eps, \
                         tc.tile_pool(name="edgep2", bufs=2) as ep2, \
                         tc.tile_pool(name="spmm_ps", bufs=2, space="PSUM") as spmm_ps, \
                         tc.tile_pool(name="zps_pool", bufs=2, space="PSUM") as zps_pool:
                        for k in range(NPIECE):
                            psb = spmm_ps.tile([128, BPP, 128], F32, tag="blkps")
                            psz = zps_pool.tile([128, BPP, 2], F32, tag="zps")
                            for p in range(2):
                                slot0 = p * C_PASS + k * PC
                                Gt = ep.tile([128, PC, 128], BF16, tag="Gt")
                                nc.gpsimd.dma_gather(
                                    out_ap=Gt[:, :, :],
                                    in_ap=table[p * SPLIT:, :],
                                    idxs_ap=gidx[:, p, k * (NP_IDX // 16):(k + 1) * (NP_IDX // 16)],
                                    num_idxs=NP_IDX, num_idxs_reg=NP_IDX, elem_size=128, single_packet=False)
                                # plain one-hot [128e, c, 128n]; built on DVE in
                                # iters 0/1, spilled to DRAM in iter 1, re-read in
                                # iter 2 (identical content across iterations).
                                oh = ep2.tile([128, PC, 128], BF16, tag="oh")
                                cache_ready = n_iters > 2 and (l > 0 or it > 1)
                                if cache_ready and not (it == 0):
                                    nc.sync.dma_start(oh[:, :, :],
                                                      ohcache[:, slot0:slot0 + PC, :])
                                else:
                                    nc.vector.tensor_tensor(
                                        oh[:, :, :],
                                        offs[:, slot0:slot0 + PC].unsqueeze(2)
                                            .broadcast_to([128, PC, 128]),
                                        iota.unsqueeze(1).broadcast_to([128, PC, 128]),
                                        OP.is_equal)
                                    if l == 0 and it == 1 and n_iters > 2:
                                        nc.sync.dma_start(ohcache[:, slot0:slot0 + PC, :],
                                                          oh[:, :, :])

                                if it == 0:
                                    idx0 = (p * NPIECE + k) * (NP_IDX // 16)
                                    idx1 = (p * NPIECE + k + 1) * (NP_IDX // 16)
                                    if l == 0:
                                        SRt = eps.tile([128, PC, 128], BF16, tag="SRt")
                                        nc.gpsimd.dma_gather(
                                            out_ap=SRt[:, :, :],
                                            in_ap=srtab[:, :],
                                            idxs_ap=tidx[:, idx0:idx1],
                                            num_idxs=NP_IDX, num_idxs_reg=NP_IDX, elem_size=128, single_packet=False)
                                        nc.vector.tensor_copy(
                                            SRx[:, slot0:slot0 + PC, :], SRt[:, :, 0:8])
                                    SHt = eps.tile([128, PC, 128], BF16, tag="SHt")
                                    nc.gpsimd.dma_gather(
                                        out_ap=SHt[:, :, :],
                                        in_ap=shtab[:, :],
                                        idxs_ap=shidx[:, idx0:idx1],
                                        num_idxs=NP_IDX, num_idxs_reg=NP_IDX, elem_size=128, single_packet=False)
                                    # scores
                                    sc = ep2.tile([128, PC, 2], F32, tag="sc")
                                    nc.vector.tensor_tensor(sc[:, :, :], Gt[:, :, 64:66],
                                                            Gt[:, :, 66:68], OP.add)
                                    t2 = ep2.tile([128, PC, 2], F32, tag="t2")
                                    nc.vector.tensor_tensor(
                                        t2[:, :, :],
                                        SRx[:, slot0:slot0 + PC, 2 * l:2 * l + 2],
                                        SRx[:, slot0:slot0 + PC, 4 + 2 * l:6 + 2 * l], OP.add)
                                    nc.vector.tensor_tensor(sc[:, :, :], sc[:, :, :], t2[:, :, :],
                                                            OP.add)
                                    nc.vector.tensor_tensor(t2[:, :, :], SHt[:, :, 0:2],
                                                            SHt[:, :, 2:4], OP.add)
                                    nc.vector.tensor_tensor(sc[:, :, :], sc[:, :, :], t2[:, :, :],
                                                            OP.add)
                                    nc.vector.scalar_tensor_tensor(
                                        sc[:, :, :], sc[:, :, :], LRELU, sc[:, :, :],
                                        OP.mult, OP.max)
                                    nc.scalar.activation(At[:, slot0:slot0 + PC, :],
                                                         sc[:, :, :], AF.Exp)

                                # messages (in place into Gt; head1 first)
                                g1 = 0 if it == 0 else 64
                                msg = Gt
                                At0 = At[:, slot0:slot0 + PC, 0].unsqueeze(2) \
                                    .broadcast_to([128, PC, 64])
                                At1 = At[:, slot0:slot0 + PC, 1].unsqueeze(2) \
                                    .broadcast_to([128, PC, 64])
                                nc.vector.tensor_tensor(msg[:, :, 64:128],
                                                        Gt[:, :, g1:g1 + 64], At1, OP.mult)
                                nc.vector.tensor_tensor(msg[:, :, 0:64], Gt[:, :, 0:64], At0,
                                                        OP.mult)
                                # spmm (+ z columns via separate 2-col matmul on iter 0)
                                # PSUM zero-regions are whole banks: start/stop once per bank.
                                bank1_j0 = 4 * CPB
                                for j in range(PC):
                                    bl = j // CPB
                                    st = (p == 0) and (j == 0 or j == bank1_j0)
                                    sp = (p == 1) and (j == bank1_j0 - 1 or j == PC - 1)
                                    nc.tensor.matmul(
                                        psb[:, bl, :], oh[:, j, :], msg[:, j, :],
                                        start=st, stop=sp)
                                    if it == 0:
                                        nc.tensor.matmul(
                                            psz[:, bl, :], oh[:, j, :],
                                            At[:, slot0 + j, :],
                                            start=(p == 0 and j == 0),
                                            stop=(p == 1 and j == PC - 1))

                            # piece epilogue: z, recip, Z assembly
                            if it == 0:
                                b0 = k * BPP
                                nc.vector.tensor_scalar(zt[:, b0:b0 + BPP, :], psz[:, :, :],
                                                        1e-30, None, OP.max)
                                nc.vector.reciprocal(recip[:, b0:b0 + BPP, :],
                                                     zt[:, b0:b0 + BPP, :])
                                nc.vector.tensor_scalar(recip[:, b0:b0 + BPP, :],
                                                        recip[:, b0:b0 + BPP, :],
                                                        1.0 - ALPHA, None, OP.mult)
                            for bl in range(BPP):
                                b = k * BPP + bl
                                nc.scalar.activation(Z[:, b, 0:64], psb[:, bl, 0:64], AF.Copy,
                                                     scale=recip[:, b, 0:1])
                                nc.scalar.activation(Z[:, b, 64:128], psb[:, bl, 64:128],
                                                     AF.Copy, scale=recip[:, b, 1:2])
                                nc.vector.scalar_tensor_tensor(
                                    Z[:, b, :].rearrange("p (o d) -> p o d", o=2),
                                    h_t[:, b, :].unsqueeze(1)
                                        .broadcast_to([128, 2, 64]),
                                    ALPHA,
                                    Z[:, b, :].rearrange("p (o d) -> p o d", o=2),
                                    OP.mult, OP.add)

                        if debug and l == 0 and it == 0:
                            nc.sync.dma_start(dbg_z0.ap().rearrange("(b p) c -> p b c", p=128),
                                              Z[:, :, :])
                            nc.sync.dma_start(
                                dbg_at.ap(),
                                At[:, :, :])

                        if it < n_iters - 1:
                            tabz = ep.tile([128, NBLK, 128], BF16, tag="Gt")
                            nc.vector.tensor_copy(tabz[:, :, :], Z[:, :, :])
                            nc.sync.dma_start(tab_in.rearrange("(b p) c -> p b c", p=128),
                                              tabz[:, :, :])
                            if do_ag:
                                nc.gpsimd.collective_compute(
                                    "AllGather", OP.bypass,
                                    replica_groups=[list(range(CORES))],
                                    ins=[tab_in.opt()], outs=[table.opt()])

                # ================= conv + residual =================
                with tc.tile_pool(name="convp", bufs=2) as cp, \
                     tc.tile_pool(name="convps", bufs=4, space="PSUM") as cps:
                    for b in range(NBLK):
                        pzt = cps.tile([128, 128], F32, tag="pzt")
                        nc.tensor.transpose(pzt[:, :], Z[:, b, :], idf[:, :])
                        Zt = cp.tile([128, 128], F32, tag="Zt")
                        nc.scalar.activation(Zt[:, :], pzt[:, :], AF.Copy)
                        pc_ = cps.tile([128, 64], F32, tag="pc")
                        nc.tensor.matmul(pc_[:, :], Zt[:, :], Wo[:, l, :],
                                         start=True, stop=True)
                        nc.vector.tensor_tensor(ent[:, b, :], pc_[:, :], ent[:, b, :], OP.add)

            nc.sync.dma_start(out_ext.ap().rearrange("(b p) f -> p b f", p=128),
                              ent[:, :, :])

    nc.compile()
    return nc


# ----------------------------------------------------------------------------
# host wrapper
# ----------------------------------------------------------------------------

def _make_in_maps(inputs, plans):
    ent = np.asarray(inputs["entity_embed"], np.float32)
    rel = np.zeros((512, DIM), np.float32)
    rel[:N_REL] = np.asarray(inputs["relation_embed"], np.float32)
    lng = np.tile(np.asarray(inputs["ln_gamma"], np.float32)[None], (128, 1, 1))
    lnb = np.tile(np.asarray(inputs["ln_beta"], np.float32)[None], (128, 1, 1))
    W = np.stack([np.asarray(inputs["W_h"], np.float32),
                  np.asarray(inputs["W_t"], np.float32),
                  np.asarray(inputs["W_r"], np.float32)], axis=0)
    att = np.stack([np.asarray(inputs["att_h"], np.float32),
                    np.asarray(inputs["att_t"], np.float32),
                    np.asarray(inputs["att_r"], np.float32)], axis=0)
    att_rep = np.tile(att.reshape(3, L_LAYERS, 1, HD), (1, 1, 128, 1)).astype(np.float32)
    Wo = np.asarray(inputs["W_o"], np.float32)
    iota = np.tile(np.arange(128, dtype=np.float32)[None], (128, 1)).astype(BF)
    idf = np.eye(128, dtype=np.float32)

    common = dict(rel_emb=rel, ln_g=np.ascontiguousarray(lng), ln_b=np.ascontiguousarray(lnb),
                  W_htr=np.ascontiguousarray(W), att_rep=np.ascontiguousarray(att_rep),
                  W_o=np.ascontiguousarray(Wo), iota=iota, ident_f=idf)
    in_maps = []
    for pl in plans:
        sl = np.zeros((M_PER_CORE, DIM), np.float32)
        nv = pl["n_valid"]
        sl[:nv] = ent[pl["node_lo"]:pl["node_lo"] + nv]
        m = dict(common)
        m["ent_slice"] = sl
        m["gidx"] = np.ascontiguousarray(np.stack(pl["gidx"], axis=1))
        m["tidx"] = pl["tidx"]
        m["shidx"] = pl["shidx"]
        m["offs"] = pl["offs"]
        in_maps.append(m)
    return in_maps


_CACHE = {}


def _get_nc(meta_key, meta, debug):
    key = (meta_key, debug)
    if key not in _CACHE:
        _CACHE[key] = _build_nc(meta, debug=debug)
    return _CACHE[key]


def run(inputs, debug=False, trace=False):
    plans, meta = _build_plan(np.asarray(inputs["edge_index"]),
                              np.asarray(inputs["edge_type"]))
    nc = _get_nc((meta["CPB"],), meta, debug)
    in_maps = _make_in_maps(inputs, plans)
    res = bass_utils.run_bass_kernel_spmd(nc, in_maps, core_ids=list(range(CORES)),
                                          trace=trace)
    out = np.zeros((N_ENT, DIM), np.float32)
    for c, pl in enumerate(plans):
        nv = pl["n_valid"]
        sl = np.asarray(res.results[c]["out_slice"])
        out[pl["node_lo"]:pl["node_lo"] + nv] = sl[:nv]
    return out, res, plans, meta


def kernel(**inputs) -> np.ndarray:
    out, _, _, _ = run(inputs)
    return out.astype(np.asarray(inputs["entity_embed"]).dtype)



# revision 2
# speedup vs baseline: 1.1718x; 1.1718x over previous
"""Trainium2 Bass kernel for nn_DAGNLinkPrediction (GNN message passing).

Self-contained: host-side integer preprocessing (sharding/permutation) + bass/tile
kernel + SPMD launch across 8 NeuronCores via run_bass_kernel_spmd.

Sharding: edges partitioned by src owner core (6272 rows/core). Within each core,
nodes are PERMUTED into 49 blocks of 128 rows so that per-(dst-half, block) edge
counts fit a fixed chunk-capacity pattern (6,5,5,5,5,5,5 per 7-block piece) —
this cuts edge-chunk padding from ~40% to ~2.5%. Per power iteration each core
gathers Z[dst] rows (256B bf16) with gpsimd.dma_gather, computes messages,
segment-sums by src via PE matmuls with 0/1 one-hot matrices, and AllGathers the
updated bf16 node table.
"""
import numpy as np
import ml_dtypes

from concourse import bass, bacc, tile, bass_utils, mybir

BF = ml_dtypes.bfloat16
F32 = mybir.dt.float32
BF16 = mybir.dt.bfloat16
I16 = mybir.dt.int16

CORES = 8
N_ENT = 50000
N_REL = 500
HEADS = 2
DIM = 64
HD = HEADS * DIM                 # 128
M_PER_CORE = 6272                # 49*128
NBLK = M_PER_CORE // 128         # 49
TAB_ROWS = CORES * M_PER_CORE    # 50176
SPLIT = TAB_ROWS // 2            # 25088 == 4*M_PER_CORE (core boundary)
NPIECE = 7                       # pieces per pass; NBLK = 7*7
BPP = NBLK // NPIECE             # blocks per piece = 7
ALPHA = 0.15
LN_EPS = 1e-5
L_LAYERS = 2
POW_ITER = 3
LRELU = 0.01

# chunk capacities per block within a piece (sums to PC chunks per piece)
CAP_PAT = (6, 5, 5, 5, 5, 5, 5)
CUM = (0, 6, 11, 16, 21, 26, 31)          # chunk offset of block-in-piece
PC = sum(CAP_PAT)                          # 36 chunks per piece
C_PASS = NPIECE * PC                       # 252 chunks per half
NIDX = C_PASS * 128                        # idx slots per half
NIDX_PIECE = PC * 128
J_B1 = CUM[4]                              # first chunk of PSUM bank 1
BLKMAP = tuple(i for i in range(BPP) for _ in range(CAP_PAT[i]))

AF = mybir.ActivationFunctionType
OP = mybir.AluOpType


# ----------------------------------------------------------------------------
# host-side preprocessing (integer/layout only)
# ----------------------------------------------------------------------------

def _wrap_idxs(idx):
    n = idx.shape[0]
    base = idx.reshape(n // 16, 16).T.astype(np.int16)
    return np.ascontiguousarray(np.tile(base, (8, 1)))


def _balance_core(deg):
    """Assign 6272 local nodes to 49 blocks s.t. per-(half, block) edge counts
    fit cap[b]*128. Returns perm: local node id -> local row."""
    caps = np.array([CAP_PAT[b % BPP] for b in range(NBLK)], np.int64)
    capn = caps * 128
    tot = deg.sum(1)
    order = np.argsort(-tot, kind="stable")
    loads = np.zeros((NBLK, 2), np.int64)
    fill = np.zeros(NBLK, np.int64)
    blk_of = np.full(M_PER_CORE, -1, np.int64)
    for n in order:
        dlo, dhi = deg[n]
        if dlo == 0 and dhi == 0:
            break
        feas = (fill < 128) & (loads[:, 0] + dlo <= capn) & (loads[:, 1] + dhi <= capn)
        if not feas.any():
            raise RuntimeError("balance infeasible")
        u = np.maximum((loads[:, 0] + dlo) / capn, (loads[:, 1] + dhi) / capn)
        u[~feas] = np.inf
        b = int(np.argmin(u))
        loads[b, 0] += dlo
        loads[b, 1] += dhi
        fill[b] += 1
        blk_of[n] = b
    # zero-degree nodes fill remaining slots
    rem = np.where(blk_of < 0)[0]
    space = np.repeat(np.arange(NBLK), (128 - fill).astype(np.int64))
    blk_of[rem] = space[: len(rem)]
    # rows within block: placement order
    perm = np.zeros(M_PER_CORE, np.int64)
    pos = np.zeros(NBLK, np.int64)
    for n in np.concatenate([order[blk_of[order] >= 0][: (tot > 0).sum()], rem]):
        b = blk_of[n]
        perm[n] = b * 128 + pos[b]
        pos[b] += 1
    assert (pos == 128).all()
    return perm


def _build_plan(edge_index, edge_type):
    src = edge_index[0].astype(np.int64)
    dst = edge_index[1].astype(np.int64)
    et = edge_type.astype(np.int64)

    half = ((dst // M_PER_CORE) >= (CORES // 2)).astype(np.int64)
    deg = np.zeros((CORES * M_PER_CORE, 2), np.int64)
    np.add.at(deg, (src, half), 1)

    perms = []
    row_global = np.zeros(CORES * M_PER_CORE, np.int64)
    for c in range(CORES):
        p = _balance_core(deg[c * M_PER_CORE:(c + 1) * M_PER_CORE])
        perms.append(p)
        row_global[c * M_PER_CORE:(c + 1) * M_PER_CORE] = c * M_PER_CORE + p

    src_row = row_global[src]          # permuted local+core row of src
    dst_row = row_global[dst]          # permuted global row of dst
    core_of = src // M_PER_CORE
    blk = (src_row % M_PER_CORE) // 128

    slot_start = np.array([(b // BPP) * PC + CUM[b % BPP] for b in range(NBLK)],
                          np.int64)

    key = (core_of * 2 + half) * NBLK + blk
    order = np.argsort(key, kind="stable")
    key_sorted = key[order]
    bounds = np.searchsorted(key_sorted, np.arange(CORES * 2 * NBLK + 1))

    plans = []
    for c in range(CORES):
        pl = {}
        pl["node_lo"] = c * M_PER_CORE
        pl["n_valid"] = max(0, min(N_ENT - c * M_PER_CORE, M_PER_CORE))
        pl["perm"] = perms[c]
        gidx, offs, typs, shls = [], [], [], []
        for p in range(2):
            eids = np.full(NIDX, -1, np.int64)
            for b in range(NBLK):
                k = (c * 2 + p) * NBLK + b
                lst = order[bounds[k]:bounds[k + 1]]
                assert len(lst) <= CAP_PAT[b % BPP] * 128
                s0 = slot_start[b] * 128
                eids[s0:s0 + len(lst)] = lst
            pad = eids < 0
            e_safe = np.where(pad, 0, eids)
            didx = dst_row[e_safe] - p * SPLIT
            didx[pad] = 0
            off = (src_row[e_safe] % 128).astype(np.int64)
            off[pad] = -1
            shl = (src_row[e_safe] % M_PER_CORE).astype(np.int64)
            shl[pad] = 0
            t = et[e_safe].copy()
            t[pad] = 0
            gidx.append(_wrap_idxs(didx.astype(np.int16)))
            offs.append(off.reshape(C_PASS, 128).T.astype(np.float32))  # [128, C_PASS]
            typs.append(t)
            shls.append(shl)
        pl["gidx"] = gidx
        pl["tidx"] = _wrap_idxs(np.concatenate(typs).astype(np.int16))
        pl["shidx"] = _wrap_idxs(np.concatenate(shls).astype(np.int16))
        pl["offs"] = np.concatenate(offs, axis=1).astype(BF)  # [128, 2*C_PASS]
        plans.append(pl)

    meta = dict(C_PASS=C_PASS, NIDX=NIDX, PIECE_CHUNKS=PC, NIDX_PIECE=NIDX_PIECE)
    return plans, meta


# ----------------------------------------------------------------------------
# device kernel
# ----------------------------------------------------------------------------

def _build_nc(meta, debug=False, n_layers=L_LAYERS, n_iters=POW_ITER, do_ag=True, do_node_ag=True):
    NP_IDX = meta["NIDX_PIECE"]        # idxs per piece
    IW = meta["NIDX"] // 16            # idx cols per pass

    nc = bacc.Bacc("TRN2", target_bir_lowering=False, debug=False,
                   num_devices=CORES)

    # ---- external inputs ----
    ent_in = nc.dram_tensor("ent_slice", [M_PER_CORE, DIM], F32, kind="ExternalInput")
    rel_in = nc.dram_tensor("rel_emb", [512, DIM], F32, kind="ExternalInput")
    lng_in = nc.dram_tensor("ln_g", [128, L_LAYERS, DIM], F32, kind="ExternalInput")
    lnb_in = nc.dram_tensor("ln_b", [128, L_LAYERS, DIM], F32, kind="ExternalInput")
    W_in = nc.dram_tensor("W_htr", [3, L_LAYERS, DIM, HD], F32, kind="ExternalInput")
    att_in = nc.dram_tensor("att_rep", [3, L_LAYERS, 128, HD], F32, kind="ExternalInput")
    Wo_in = nc.dram_tensor("W_o", [L_LAYERS, HD, DIM], F32, kind="ExternalInput")
    gidx_in = nc.dram_tensor("gidx", [128, 2, IW], I16, kind="ExternalInput")
    tidx_in = nc.dram_tensor("tidx", [128, 2 * IW], I16, kind="ExternalInput")
    shidx_in = nc.dram_tensor("shidx", [128, 2 * IW], I16, kind="ExternalInput")
    offs_in = nc.dram_tensor("offs", [128, 2 * C_PASS], BF16, kind="ExternalInput")
    iota_in = nc.dram_tensor("iota", [128, 128], BF16, kind="ExternalInput")
    idf_in = nc.dram_tensor("ident_f", [128, 128], F32, kind="ExternalInput")

    out_ext = nc.dram_tensor("out_slice", [M_PER_CORE, DIM], F32, kind="ExternalOutput")
    if debug:
        dbg_at = nc.dram_tensor("dbg_at", [128, 2 * C_PASS, 2], BF16, kind="ExternalOutput")
        dbg_z0 = nc.dram_tensor("dbg_z0", [M_PER_CORE, HD], F32, kind="ExternalOutput")
        dbg_h = nc.dram_tensor("dbg_h", [M_PER_CORE, DIM], F32, kind="ExternalOutput")

    with tile.TileContext(nc) as tc:
        with tc.tile_pool(name="dram", bufs=1, space="DRAM") as dram, \
             tc.tile_pool(name="persist", bufs=1) as pp:
            table = dram.tile([TAB_ROWS, 128], BF16, tag="table")
            tab_in = dram.tile([M_PER_CORE, 128], BF16, tag="tab_in")
            srtab = dram.tile([512, 128], BF16, tag="srtab")
            shtab = dram.tile([M_PER_CORE, 128], BF16, tag="shtab")

            ent = pp.tile([128, NBLK, DIM], F32, tag="ent")
            h_t = pp.tile([128, NBLK, DIM], F32, tag="h")
            recip = pp.tile([128, NBLK, 2], F32, tag="recip")
            zt = pp.tile([128, NBLK, 2], F32, tag="zt")
            At = pp.tile([128, 2 * C_PASS, 2], BF16, tag="At")
            SRx = pp.tile([128, 2 * C_PASS, 8], BF16, tag="SRx")
            Z = pp.tile([128, NBLK, HD], F32, tag="Z")
            gidx = pp.tile([128, 2, IW], I16, tag="gidx")
            tidx = pp.tile([128, 2 * IW], I16, tag="tidx")
            shidx = pp.tile([128, 2 * IW], I16, tag="shidx")
            offs = pp.tile([128, 2 * C_PASS], BF16, tag="offs")
            iota = pp.tile([128, 128], BF16, tag="iota")
            idf = pp.tile([128, 128], F32, tag="idf")
            lng = pp.tile([128, L_LAYERS, DIM], F32, tag="lng")
            lnb = pp.tile([128, L_LAYERS, DIM], F32, tag="lnb")
            Wht = pp.tile([64, 3 * L_LAYERS, HD], F32, tag="Wht")
            attr = pp.tile([128, 3 * L_LAYERS, HD], F32, tag="attr")
            Wo = pp.tile([HD, L_LAYERS, DIM], F32, tag="Wo")

            # ---- load inputs ----
            nc.sync.dma_start(ent[:, :, :], ent_in.ap().rearrange("(b p) f -> p b f", p=128))
            nc.sync.dma_start(gidx[:, :, :], gidx_in.ap())
            nc.sync.dma_start(tidx[:, :], tidx_in.ap())
            nc.sync.dma_start(shidx[:, :], shidx_in.ap())
            nc.sync.dma_start(offs[:, :], offs_in.ap())
            nc.sync.dma_start(iota[:, :], iota_in.ap())
            nc.sync.dma_start(idf[:, :], idf_in.ap())
            nc.sync.dma_start(lng[:, :, :], lng_in.ap())
            nc.sync.dma_start(lnb[:, :, :], lnb_in.ap())
            nc.sync.dma_start(Wht[:, :, :], W_in.ap().rearrange("r l k m -> k (r l) m"))
            nc.sync.dma_start(attr[:, :, :], att_in.ap().rearrange("r l p m -> p (r l) m"))
            nc.sync.dma_start(Wo[:, :, :], Wo_in.ap().rearrange("l k m -> k l m"))

            def node_scores_block(pool, psum, lhsT, Wslice, att_ap, out_ap):
                """tanh(x@W) . att summed over d -> out_ap [128,2] (f32)."""
                pt = psum.tile([128, HD], F32, tag="ns_ps")
                nc.tensor.matmul(pt[:, :], lhsT, Wslice, start=True, stop=True)
                tt = pool.tile([128, HD], F32, tag="ns_tt")
                nc.scalar.activation(tt[:, :], pt[:, :], AF.Tanh)
                tm = pool.tile([128, HD], F32, tag="ns_tm")
                nc.vector.tensor_tensor(tm[:, :], tt[:, :], att_ap, OP.mult)
                nc.vector.tensor_reduce(out_ap, tm.rearrange("p (h d) -> p h d", h=2),
                                        mybir.AxisListType.X, OP.add)

            for l in range(n_layers):
                # ================= node phase =================
                with tc.tile_pool(name="nodep", bufs=2) as np_pool, \
                     tc.tile_pool(name="nodebig", bufs=1) as np_big, \
                     tc.tile_pool(name="nodeps", bufs=2, space="PSUM") as np_psum:
                    # layernorm
                    x = ent
                    mu = np_pool.tile([128, NBLK], F32, tag="mu")
                    nc.vector.tensor_reduce(mu[:, :], x[:, :, :], mybir.AxisListType.X, OP.add)
                    nc.vector.tensor_scalar(mu[:, :], mu[:, :], 1.0 / DIM, None, OP.mult)
                    xc = np_big.tile([128, NBLK, DIM], F32, tag="xc")
                    nc.vector.tensor_tensor(
                        xc[:, :, :], x[:, :, :],
                        mu.unsqueeze(2).broadcast_to([128, NBLK, DIM]), OP.subtract)
                    sq = np_big.tile([128, NBLK, DIM], F32, tag="sq")
                    nc.vector.tensor_tensor(sq[:, :, :], xc[:, :, :], xc[:, :, :], OP.mult)
                    var = np_pool.tile([128, NBLK], F32, tag="var")
                    nc.vector.tensor_reduce(var[:, :], sq[:, :, :], mybir.AxisListType.X, OP.add)
                    nc.vector.tensor_scalar(var[:, :], var[:, :], 1.0 / DIM, LN_EPS, OP.mult, OP.add)
                    std = np_pool.tile([128, NBLK], F32, tag="std")
                    nc.scalar.activation(std[:, :], var[:, :], AF.Sqrt)
                    rstd = np_pool.tile([128, NBLK], F32, tag="rstd")
                    nc.vector.reciprocal(rstd[:, :], std[:, :])
                    nc.vector.tensor_tensor(
                        h_t[:, :, :], xc[:, :, :],
                        rstd.unsqueeze(2).broadcast_to([128, NBLK, DIM]), OP.mult)
                    nc.vector.tensor_tensor(
                        h_t[:, :, :], h_t[:, :, :],
                        lng[:, l, :].unsqueeze(1).broadcast_to([128, NBLK, DIM]),
                        OP.mult)
                    nc.vector.tensor_tensor(
                        h_t[:, :, :], h_t[:, :, :],
                        lnb[:, l, :].unsqueeze(1).broadcast_to([128, NBLK, DIM]),
                        OP.add)
                    if debug and l == 0:
                        nc.sync.dma_start(dbg_h.ap().rearrange("(b p) f -> p b f", p=128),
                                          h_t[:, :, :])

                    # transpose h -> ht [64, b, 128]
                    ht = np_big.tile([64, NBLK, 128], F32, tag="ht")
                    for b in range(NBLK):
                        ps = np_psum.tile([64, 128], F32, tag="trh")
                        nc.tensor.transpose(ps[:, :], h_t[:, b, :], idf[:, :])
                        nc.scalar.activation(ht[:, b, :], ps[:, :], AF.Copy)

                    # s_h, s_t  [128, NBLK, 2] f32
                    s_f0 = np_pool.tile([128, NBLK, 2], F32, tag="s_f0")
                    s_f1 = np_pool.tile([128, NBLK, 2], F32, tag="s_f1")
                    s_f = [s_f0, s_f1]
                    for r in range(2):
                        for b in range(NBLK):
                            node_scores_block(np_pool, np_psum, ht[:, b, :],
                                              Wht[:, r * L_LAYERS + l, :],
                                              attr[:, r * L_LAYERS + l, :],
                                              s_f[r][:, b, :])
                    # sh table rows: [bf16(s_h)(2), bf16(residual)(2), 0...]
                    shsb = np_big.tile([128, NBLK, 128], BF16, tag="shsb")
                    shm_f = np_pool.tile([128, NBLK, 2], F32, tag="shm_f")
                    nc.vector.memset(shsb[:, :, 4:128], 0.0)
                    nc.vector.tensor_copy(shsb[:, :, 0:2], s_f[0][:, :, :])
                    nc.vector.tensor_copy(shm_f[:, :, :], shsb[:, :, 0:2])
                    nc.vector.tensor_tensor(shsb[:, :, 2:4], s_f[0][:, :, :], shm_f[:, :, :],
                                            OP.subtract)
                    nc.sync.dma_start(shtab.rearrange("(b p) c -> p b c", p=128),
                                      shsb[:, :, :])

                    # gather table slice: [h | s_t | s_t_res | 1 | 0...]
                    tabsb = np_big.tile([128, NBLK, 128], BF16, tag="tabsb")
                    nc.vector.tensor_copy(tabsb[:, :, 0:64], h_t[:, :, :])
                    nc.vector.tensor_copy(tabsb[:, :, 64:66], s_f[1][:, :, :])
                    stm_f = np_pool.tile([128, NBLK, 2], F32, tag="stm_f")
                    nc.vector.tensor_copy(stm_f[:, :, :], tabsb[:, :, 64:66])
                    nc.vector.tensor_tensor(tabsb[:, :, 66:68], s_f[1][:, :, :], stm_f[:, :, :],
                                            OP.subtract)
                    nc.vector.memset(tabsb[:, :, 68:69], 1.0)
                    nc.vector.memset(tabsb[:, :, 69:128], 0.0)
                    nc.sync.dma_start(tab_in.rearrange("(b p) c -> p b c", p=128),
                                      tabsb[:, :, :])
                    if do_node_ag:
                        nc.gpsimd.collective_compute(
                            "AllGather", OP.bypass,
                            replica_groups=[list(range(CORES))],
                            ins=[tab_in.opt()], outs=[table.opt()])
                    else:
                        nc.sync.dma_start(table[0:M_PER_CORE, :],
                                          tab_in[:, :])

                    # s_r table (once, both layers)
                    if l == 0:
                        relsb = np_pool.tile([128, 4, DIM], F32, tag="relsb")
                        nc.sync.dma_start(relsb[:, :, :],
                                          rel_in.ap().rearrange("(b p) f -> p b f", p=128))
                        relt = np_pool.tile([64, 4, 128], F32, tag="relt")
                        for b in range(4):
                            ps = np_psum.tile([64, 128], F32, tag="trh")
                            nc.tensor.transpose(ps[:, :], relsb[:, b, :], idf[:, :])
                            nc.scalar.activation(relt[:, b, :], ps[:, :], AF.Copy)
                        srsb = np_pool.tile([128, 4, 128], BF16, tag="srsb")
                        nc.vector.memset(srsb[:, :, :], 0.0)
                        sr_f = np_pool.tile([128, 4, 2 * L_LAYERS], F32, tag="sr_f")
                        for ll in range(L_LAYERS):
                            for b in range(4):
                                node_scores_block(np_pool, np_psum, relt[:, b, :],
                                                  Wht[:, 2 * L_LAYERS + ll, :],
                                                  attr[:, 2 * L_LAYERS + ll, :],
                                                  sr_f[:, b, 2 * ll:2 * ll + 2])
                        nc.vector.tensor_copy(srsb[:, :, 0:4], sr_f[:, :, :])
                        srm_f = np_pool.tile([128, 4, 4], F32, tag="srm_f")
                        nc.vector.tensor_copy(srm_f[:, :, :], srsb[:, :, 0:4])
                        nc.vector.tensor_tensor(srsb[:, :, 4:8], sr_f[:, :, :], srm_f[:, :, :],
                                                OP.subtract)
                        nc.sync.dma_start(srtab.rearrange("(b p) c -> p b c", p=128),
                                          srsb[:, :, :])

                # ================= edge phase =================
                for it in range(n_iters):
                    with tc.tile_pool(name="edgep", bufs=3) as ep, \
                         tc.tile_pool(name="edgeps", bufs=2) as eps, \
                         tc.tile_pool(name="edgep2", bufs=2) as ep2, \
                         tc.tile_pool(name="spmm_ps", bufs=2, space="PSUM") as spmm_ps, \
                         tc.tile_pool(name="zps_pool", bufs=2, space="PSUM") as zps_pool:
                        for k in range(NPIECE):
                            psb = spmm_ps.tile([128, BPP, 128], F32, tag="blkps")
                            psz = zps_pool.tile([128, BPP, 2], F32, tag="zps")
                            for p in range(2):
                                slot0 = p * C_PASS + k * PC
                                Gt = ep.tile([128, PC, 128], BF16, tag="Gt")
                                nc.gpsimd.dma_gather(
                                    out_ap=Gt[:, :, :],
                                    in_ap=table[p * SPLIT:, :],
                                    idxs_ap=gidx[:, p, k * (NP_IDX // 16):(k + 1) * (NP_IDX // 16)],
                                    num_idxs=NP_IDX, num_idxs_reg=NP_IDX, elem_size=128, single_packet=False)
                                # plain one-hot [128e, c, 128n] built on DVE
                                oh = ep2.tile([128, PC, 128], BF16, tag="oh")
                                nc.vector.tensor_tensor(
                                    oh[:, :, :],
                                    offs[:, slot0:slot0 + PC].unsqueeze(2)
                                        .broadcast_to([128, PC, 128]),
                                    iota.unsqueeze(1).broadcast_to([128, PC, 128]),
                                    OP.is_equal)

                                if it == 0:
                                    idx0 = (p * NPIECE + k) * (NP_IDX // 16)
                                    idx1 = (p * NPIECE + k + 1) * (NP_IDX // 16)
                                    if l == 0:
                                        SRt = eps.tile([128, PC, 128], BF16, tag="SRt")
                                        nc.gpsimd.dma_gather(
                                            out_ap=SRt[:, :, :],
                                            in_ap=srtab[:, :],
                                            idxs_ap=tidx[:, idx0:idx1],
                                            num_idxs=NP_IDX, num_idxs_reg=NP_IDX, elem_size=128, single_packet=False)
                                        nc.vector.tensor_copy(
                                            SRx[:, slot0:slot0 + PC, :], SRt[:, :, 0:8])
                                    SHt = eps.tile([128, PC, 128], BF16, tag="SHt")
                                    nc.gpsimd.dma_gather(
                                        out_ap=SHt[:, :, :],
                                        in_ap=shtab[:, :],
                                        idxs_ap=shidx[:, idx0:idx1],
                                        num_idxs=NP_IDX, num_idxs_reg=NP_IDX, elem_size=128, single_packet=False)
                                    # scores
                                    sc = ep2.tile([128, PC, 2], F32, tag="sc")
                                    nc.vector.tensor_tensor(sc[:, :, :], Gt[:, :, 64:66],
                                                            Gt[:, :, 66:68], OP.add)
                                    t2 = ep2.tile([128, PC, 2], F32, tag="t2")
                                    nc.vector.tensor_tensor(
                                        t2[:, :, :],
                                        SRx[:, slot0:slot0 + PC, 2 * l:2 * l + 2],
                                        SRx[:, slot0:slot0 + PC, 4 + 2 * l:6 + 2 * l], OP.add)
                                    nc.vector.tensor_tensor(sc[:, :, :], sc[:, :, :], t2[:, :, :],
                                                            OP.add)
                                    nc.vector.tensor_tensor(t2[:, :, :], SHt[:, :, 0:2],
                                                            SHt[:, :, 2:4], OP.add)
                                    nc.vector.tensor_tensor(sc[:, :, :], sc[:, :, :], t2[:, :, :],
                                                            OP.add)
                                    nc.vector.scalar_tensor_tensor(
                                        sc[:, :, :], sc[:, :, :], LRELU, sc[:, :, :],
                                        OP.mult, OP.max)
                                    nc.scalar.activation(At[:, slot0:slot0 + PC, :],
                                                         sc[:, :, :], AF.Exp)

                                # messages (in place into Gt; head1 first)
                                g1 = 0 if it == 0 else 64
                                msg = Gt
                                At0 = At[:, slot0:slot0 + PC, 0].unsqueeze(2) \
                                    .broadcast_to([128, PC, 64])
                                At1 = At[:, slot0:slot0 + PC, 1].unsqueeze(2) \
                                    .broadcast_to([128, PC, 64])
                                nc.vector.tensor_tensor(msg[:, :, 64:128],
                                                        Gt[:, :, g1:g1 + 64], At1, OP.mult)
                                nc.vector.tensor_tensor(msg[:, :, 0:64], Gt[:, :, 0:64], At0,
                                                        OP.mult)
                                # spmm (+ z columns via separate 2-col matmul on iter 0)
                                # PSUM zero-regions are whole banks: start/stop once per bank.
                                for j in range(PC):
                                    bl = BLKMAP[j]
                                    st = (p == 0) and (j == 0 or j == J_B1)
                                    sp = (p == 1) and (j == J_B1 - 1 or j == PC - 1)
                                    nc.tensor.matmul(
                                        psb[:, bl, :], oh[:, j, :], msg[:, j, :],
                                        start=st, stop=sp)
                                    if it == 0:
                                        nc.tensor.matmul(
                                            psz[:, bl, :], oh[:, j, :],
                                            At[:, slot0 + j, :],
                                            start=(p == 0 and j == 0),
                                            stop=(p == 1 and j == PC - 1))

                            # piece epilogue: z, recip, Z assembly
                            if it == 0:
                                b0 = k * BPP
                                nc.vector.tensor_scalar(zt[:, b0:b0 + BPP, :], psz[:, :, :],
                                                        1e-30, None, OP.max)
                                nc.vector.reciprocal(recip[:, b0:b0 + BPP, :],
                                                     zt[:, b0:b0 + BPP, :])
                                nc.vector.tensor_scalar(recip[:, b0:b0 + BPP, :],
                                                        recip[:, b0:b0 + BPP, :],
                                                        1.0 - ALPHA, None, OP.mult)
                            for bl in range(BPP):
                                b = k * BPP + bl
                                nc.scalar.activation(Z[:, b, 0:64], psb[:, bl, 0:64], AF.Copy,
                                                     scale=recip[:, b, 0:1])
                                nc.scalar.activation(Z[:, b, 64:128], psb[:, bl, 64:128],
                                                     AF.Copy, scale=recip[:, b, 1:2])
                                nc.vector.scalar_tensor_tensor(
                                    Z[:, b, :].rearrange("p (o d) -> p o d", o=2),
                                    h_t[:, b, :].unsqueeze(1)
                                        .broadcast_to([128, 2, 64]),
                                    ALPHA,
                                    Z[:, b, :].rearrange("p (o d) -> p o d", o=2),
                                    OP.mult, OP.add)

                        if debug and l == 0 and it == 0:
                            nc.sync.dma_start(dbg_z0.ap().rearrange("(b p) c -> p b c", p=128),
                                              Z[:, :, :])
                            nc.sync.dma_start(
                                dbg_at.ap(),
                                At[:, :, :])

                        if it < n_iters - 1:
                            tabz = ep.tile([128, NBLK, 128], BF16, tag="Gt")
                            nc.vector.tensor_copy(tabz[:, :, :], Z[:, :, :])
                            nc.sync.dma_start(tab_in.rearrange("(b p) c -> p b c", p=128),
                                              tabz[:, :, :])
                            if do_ag:
                                nc.gpsimd.collective_compute(
                                    "AllGather", OP.bypass,
                                    replica_groups=[list(range(CORES))],
                                    ins=[tab_in.opt()], outs=[table.opt()])

                # ================= conv + residual =================
                with tc.tile_pool(name="convp", bufs=2) as cp, \
                     tc.tile_pool(name="convps", bufs=4, space="PSUM") as cps:
                    for b in range(NBLK):
                        pzt = cps.tile([128, 128], F32, tag="pzt")
                        nc.tensor.transpose(pzt[:, :], Z[:, b, :], idf[:, :])
                        Zt = cp.tile([128, 128], F32, tag="Zt")
                        nc.scalar.activation(Zt[:, :], pzt[:, :], AF.Copy)
                        pc_ = cps.tile([128, 64], F32, tag="pc")
                        nc.tensor.matmul(pc_[:, :], Zt[:, :], Wo[:, l, :],
                                         start=True, stop=True)
                        nc.vector.tensor_tensor(ent[:, b, :], pc_[:, :], ent[:, b, :], OP.add)

            nc.sync.dma_start(out_ext.ap().rearrange("(b p) f -> p b f", p=128),
                              ent[:, :, :])

    nc.compile()
    return nc


# ----------------------------------------------------------------------------
# host wrapper
# ----------------------------------------------------------------------------

def _make_in_maps(inputs, plans):
    ent = np.asarray(inputs["entity_embed"], np.float32)
    rel = np.zeros((512, DIM), np.float32)
    rel[:N_REL] = np.asarray(inputs["relation_embed"], np.float32)
    lng = np.tile(np.asarray(inputs["ln_gamma"], np.float32)[None], (128, 1, 1))
    lnb = np.tile(np.asarray(inputs["ln_beta"], np.float32)[None], (128, 1, 1))
    W = np.stack([np.asarray(inputs["W_h"], np.float32),
                  np.asarray(inputs["W_t"], np.float32),
                  np.asarray(inputs["W_r"], np.float32)], axis=0)
    att = np.stack([np.asarray(inputs["att_h"], np.float32),
                    np.asarray(inputs["att_t"], np.float32),
                    np.asarray(inputs["att_r"], np.float32)], axis=0)
    att_rep = np.tile(att.reshape(3, L_LAYERS, 1, HD), (1, 1, 128, 1)).astype(np.float32)
    Wo = np.asarray(inputs["W_o"], np.float32)
    iota = np.tile(np.arange(128, dtype=np.float32)[None], (128, 1)).astype(BF)
    idf = np.eye(128, dtype=np.float32)

    common = dict(rel_emb=rel, ln_g=np.ascontiguousarray(lng), ln_b=np.ascontiguousarray(lnb),
                  W_htr=np.ascontiguousarray(W), att_rep=np.ascontiguousarray(att_rep),
                  W_o=np.ascontiguousarray(Wo), iota=iota, ident_f=idf)
    in_maps = []
    for pl in plans:
        sl = np.zeros((M_PER_CORE, DIM), np.float32)
        nv = pl["n_valid"]
        sl[pl["perm"][:nv]] = ent[pl["node_lo"]:pl["node_lo"] + nv]
        m = dict(common)
        m["ent_slice"] = sl
        m["gidx"] = np.ascontiguousarray(np.stack(pl["gidx"], axis=1))
        m["tidx"] = pl["tidx"]
        m["shidx"] = pl["shidx"]
        m["offs"] = pl["offs"]
        in_maps.append(m)
    return in_maps


_CACHE = {}


def _get_nc(meta_key, meta, debug):
    key = (meta_key, debug)
    if key not in _CACHE:
        _CACHE[key] = _build_nc(meta, debug=debug)
    return _CACHE[key]


def run(inputs, debug=False, trace=False):
    plans, meta = _build_plan(np.asarray(inputs["edge_index"]),
                              np.asarray(inputs["edge_type"]))
    nc = _get_nc((meta["C_PASS"],), meta, debug)
    in_maps = _make_in_maps(inputs, plans)
    res = bass_utils.run_bass_kernel_spmd(nc, in_maps, core_ids=list(range(CORES)),
                                          trace=trace)
    out = np.zeros((N_ENT, DIM), np.float32)
    for c, pl in enumerate(plans):
        nv = pl["n_valid"]
        sl = np.asarray(res.results[c]["out_slice"])
        out[pl["node_lo"]:pl["node_lo"] + nv] = sl[pl["perm"][:nv]]
    return out, res, plans, meta


def kernel(**inputs) -> np.ndarray:
    out, _, _, _ = run(inputs)
    return out.astype(np.asarray(inputs["entity_embed"]).dtype)


# revision 14
# speedup vs baseline: 1.1809x; 1.0077x over previous
"""Trainium2 Bass kernel for nn_DAGNLinkPrediction (GNN message passing).

Self-contained: host-side integer preprocessing (sharding/permutation) + bass/tile
kernel + SPMD launch across 8 NeuronCores via run_bass_kernel_spmd.

Sharding: edges partitioned by src owner core (6272 rows/core). Within each core,
nodes are PERMUTED into 49 blocks of 128 rows so that per-(dst-half, block) edge
counts fit a fixed chunk-capacity pattern (6,5,5,5,5,5,5 per 7-block piece) —
this cuts edge-chunk padding from ~40% to ~2.5%. Per power iteration each core
gathers Z[dst] rows (256B bf16) with gpsimd.dma_gather, computes messages,
segment-sums by src via PE matmuls with 0/1 one-hot matrices, and AllGathers the
updated bf16 node table.
"""
import numpy as np
import ml_dtypes

from concourse import bass, bacc, tile, bass_utils, mybir

BF = ml_dtypes.bfloat16
F32 = mybir.dt.float32
BF16 = mybir.dt.bfloat16
I16 = mybir.dt.int16

CORES = 8
N_ENT = 50000
N_REL = 500
HEADS = 2
DIM = 64
HD = HEADS * DIM                 # 128
M_PER_CORE = 6272                # 49*128
NBLK = M_PER_CORE // 128         # 49
TAB_ROWS = CORES * M_PER_CORE    # 50176
SPLIT = TAB_ROWS // 2            # 25088 == 4*M_PER_CORE (core boundary)
NPIECE = 7                       # pieces per pass; NBLK = 7*7
BPP = NBLK // NPIECE             # blocks per piece = 7
ALPHA = 0.15
LN_EPS = 1e-5
L_LAYERS = 2
POW_ITER = 3
LRELU = 0.01

# chunk capacities per block within a piece (sums to PC chunks per piece)
CAP_PAT = (6, 5, 5, 5, 5, 5, 5)
CUM = (0, 6, 11, 16, 21, 26, 31)          # chunk offset of block-in-piece
PC = sum(CAP_PAT)                          # 36 chunks per piece
C_PASS = NPIECE * PC                       # 252 chunks per half
NIDX = C_PASS * 128                        # idx slots per half
NIDX_PIECE = PC * 128
J_B1 = CUM[4]                              # first chunk of PSUM bank 1
BLKMAP = tuple(i for i in range(BPP) for _ in range(CAP_PAT[i]))

AF = mybir.ActivationFunctionType
OP = mybir.AluOpType


# ----------------------------------------------------------------------------
# host-side preprocessing (integer/layout only)
# ----------------------------------------------------------------------------

def _wrap_idxs(idx):
    n = idx.shape[0]
    base = idx.reshape(n // 16, 16).T.astype(np.int16)
    return np.ascontiguousarray(np.tile(base, (8, 1)))


def _balance_core(deg):
    """Assign 6272 local nodes to 49 blocks s.t. per-(half, block) edge counts
    fit cap[b]*128. Returns perm: local node id -> local row."""
    caps = np.array([CAP_PAT[b % BPP] for b in range(NBLK)], np.int64)
    capn = caps * 128
    tot = deg.sum(1)
    order = np.argsort(-tot, kind="stable")
    loads = np.zeros((NBLK, 2), np.int64)
    fill = np.zeros(NBLK, np.int64)
    blk_of = np.full(M_PER_CORE, -1, np.int64)
    for n in order:
        dlo, dhi = deg[n]
        if dlo == 0 and dhi == 0:
            break
        feas = (fill < 128) & (loads[:, 0] + dlo <= capn) & (loads[:, 1] + dhi <= capn)
        if not feas.any():
            raise RuntimeError("balance infeasible")
        u = np.maximum((loads[:, 0] + dlo) / capn, (loads[:, 1] + dhi) / capn)
        u[~feas] = np.inf
        b = int(np.argmin(u))
        loads[b, 0] += dlo
        loads[b, 1] += dhi
        fill[b] += 1
        blk_of[n] = b
    # zero-degree nodes fill remaining slots
    rem = np.where(blk_of < 0)[0]
    space = np.repeat(np.arange(NBLK), (128 - fill).astype(np.int64))
    blk_of[rem] = space[: len(rem)]
    # rows within block: placement order
    perm = np.zeros(M_PER_CORE, np.int64)
    pos = np.zeros(NBLK, np.int64)
    for n in np.concatenate([order[blk_of[order] >= 0][: (tot > 0).sum()], rem]):
        b = blk_of[n]
        perm[n] = b * 128 + pos[b]
        pos[b] += 1
    assert (pos == 128).all()
    return perm


def _build_plan(edge_index, edge_type):
    src = edge_index[0].astype(np.int64)
    dst = edge_index[1].astype(np.int64)
    et = edge_type.astype(np.int64)

    half = ((dst // M_PER_CORE) >= (CORES // 2)).astype(np.int64)
    deg = np.zeros((CORES * M_PER_CORE, 2), np.int64)
    np.add.at(deg, (src, half), 1)

    perms = []
    row_global = np.zeros(CORES * M_PER_CORE, np.int64)
    for c in range(CORES):
        p = _balance_core(deg[c * M_PER_CORE:(c + 1) * M_PER_CORE])
        perms.append(p)
        row_global[c * M_PER_CORE:(c + 1) * M_PER_CORE] = c * M_PER_CORE + p

    src_row = row_global[src]          # permuted local+core row of src
    dst_row = row_global[dst]          # permuted global row of dst
    core_of = src // M_PER_CORE
    blk = (src_row % M_PER_CORE) // 128

    slot_start = np.array([(b // BPP) * PC + CUM[b % BPP] for b in range(NBLK)],
                          np.int64)

    key = (core_of * 2 + half) * NBLK + blk
    order = np.argsort(key, kind="stable")
    key_sorted = key[order]
    bounds = np.searchsorted(key_sorted, np.arange(CORES * 2 * NBLK + 1))

    plans = []
    for c in range(CORES):
        pl = {}
        pl["node_lo"] = c * M_PER_CORE
        pl["n_valid"] = max(0, min(N_ENT - c * M_PER_CORE, M_PER_CORE))
        pl["perm"] = perms[c]
        gidx, offs, typs, shls = [], [], [], []
        for p in range(2):
            eids = np.full(NIDX, -1, np.int64)
            for b in range(NBLK):
                k = (c * 2 + p) * NBLK + b
                lst = order[bounds[k]:bounds[k + 1]]
                assert len(lst) <= CAP_PAT[b % BPP] * 128
                s0 = slot_start[b] * 128
                eids[s0:s0 + len(lst)] = lst
            pad = eids < 0
            e_safe = np.where(pad, 0, eids)
            didx = dst_row[e_safe] - p * SPLIT
            didx[pad] = 0
            off = (src_row[e_safe] % 128).astype(np.int64)
            off[pad] = -1
            shl = (src_row[e_safe] % M_PER_CORE).astype(np.int64)
            shl[pad] = 0
            t = et[e_safe].copy()
            t[pad] = 0
            gidx.append(_wrap_idxs(didx.astype(np.int16)))
            offs.append(off.reshape(C_PASS, 128).T.astype(np.float32))  # [128, C_PASS]
            typs.append(t)
            shls.append(shl)
        pl["gidx"] = gidx
        pl["tidx"] = _wrap_idxs(np.concatenate(typs).astype(np.int16))
        pl["shidx"] = _wrap_idxs(np.concatenate(shls).astype(np.int16))
        pl["offs"] = np.concatenate(offs, axis=1).astype(BF)  # [128, 2*C_PASS]
        plans.append(pl)

    meta = dict(C_PASS=C_PASS, NIDX=NIDX, PIECE_CHUNKS=PC, NIDX_PIECE=NIDX_PIECE)
    return plans, meta


# ----------------------------------------------------------------------------
# device kernel
# ----------------------------------------------------------------------------

def _build_nc(meta, debug=False, n_layers=L_LAYERS, n_iters=POW_ITER, do_ag=True, do_node_ag=True):
    NP_IDX = meta["NIDX_PIECE"]        # idxs per piece
    IW = meta["NIDX"] // 16            # idx cols per pass

    nc = bacc.Bacc("TRN2", target_bir_lowering=False, debug=False,
                   num_devices=CORES)

    # ---- external inputs ----
    ent_in = nc.dram_tensor("ent_slice", [M_PER_CORE, DIM], F32, kind="ExternalInput")
    rel_in = nc.dram_tensor("rel_emb", [512, DIM], F32, kind="ExternalInput")
    lng_in = nc.dram_tensor("ln_g", [128, L_LAYERS, DIM], F32, kind="ExternalInput")
    lnb_in = nc.dram_tensor("ln_b", [128, L_LAYERS, DIM], F32, kind="ExternalInput")
    W_in = nc.dram_tensor("W_htr", [3, L_LAYERS, DIM, HD], F32, kind="ExternalInput")
    att_in = nc.dram_tensor("att_rep", [3, L_LAYERS, 128, HD], F32, kind="ExternalInput")
    Wo_in = nc.dram_tensor("W_o", [L_LAYERS, HD, DIM], F32, kind="ExternalInput")
    gidx_in = nc.dram_tensor("gidx", [128, 2, IW], I16, kind="ExternalInput")
    tidx_in = nc.dram_tensor("tidx", [128, 2 * IW], I16, kind="ExternalInput")
    shidx_in = nc.dram_tensor("shidx", [128, 2 * IW], I16, kind="ExternalInput")
    offs_in = nc.dram_tensor("offs", [128, 2 * C_PASS], BF16, kind="ExternalInput")
    iota_in = nc.dram_tensor("iota_exp", [128, 128, PC], BF16, kind="ExternalInput")
    idf_in = nc.dram_tensor("ident_f", [128, 128], F32, kind="ExternalInput")

    out_ext = nc.dram_tensor("out_slice", [M_PER_CORE, DIM], F32, kind="ExternalOutput")
    if debug:
        dbg_at = nc.dram_tensor("dbg_at", [128, 2 * C_PASS, 2], BF16, kind="ExternalOutput")
        dbg_z0 = nc.dram_tensor("dbg_z0", [M_PER_CORE, HD], F32, kind="ExternalOutput")
        dbg_h = nc.dram_tensor("dbg_h", [M_PER_CORE, DIM], F32, kind="ExternalOutput")

    with tile.TileContext(nc) as tc:
        with tc.tile_pool(name="dram", bufs=1, space="DRAM") as dram, \
             tc.tile_pool(name="persist", bufs=1) as pp:
            table = dram.tile([TAB_ROWS, 128], BF16, tag="table")
            tab_in = dram.tile([M_PER_CORE, 128], BF16, tag="tab_in")
            srtab = dram.tile([512, 128], BF16, tag="srtab")
            shtab = dram.tile([M_PER_CORE, 128], BF16, tag="shtab")

            ent = pp.tile([128, NBLK, DIM], F32, tag="ent")
            h_t = pp.tile([128, NBLK, DIM], F32, tag="h")
            recip = pp.tile([128, NBLK, 2], F32, tag="recip")
            zt = pp.tile([128, NBLK, 2], F32, tag="zt")
            At = pp.tile([128, 2 * C_PASS, 2], BF16, tag="At")
            SRx = pp.tile([128, 2 * C_PASS, 8], BF16, tag="SRx")
            Z = pp.tile([128, NBLK, HD], F32, tag="Z")
            gidx = pp.tile([128, 2, IW], I16, tag="gidx")
            tidx = pp.tile([128, 2 * IW], I16, tag="tidx")
            shidx = pp.tile([128, 2 * IW], I16, tag="shidx")
            offs = pp.tile([128, 2 * C_PASS], BF16, tag="offs")
            iota = pp.tile([128, 128, PC], BF16, tag="iota")
            idf = pp.tile([128, 128], F32, tag="idf")
            lng = pp.tile([128, L_LAYERS, DIM], F32, tag="lng")
            lnb = pp.tile([128, L_LAYERS, DIM], F32, tag="lnb")
            Wht = pp.tile([64, 3 * L_LAYERS, HD], F32, tag="Wht")
            attr = pp.tile([128, 3 * L_LAYERS, HD], F32, tag="attr")
            Wo = pp.tile([HD, L_LAYERS, DIM], F32, tag="Wo")

            # ---- load inputs ----
            nc.sync.dma_start(ent[:, :, :], ent_in.ap().rearrange("(b p) f -> p b f", p=128))
            nc.sync.dma_start(gidx[:, :, :], gidx_in.ap())
            nc.sync.dma_start(tidx[:, :], tidx_in.ap())
            nc.sync.dma_start(shidx[:, :], shidx_in.ap())
            nc.sync.dma_start(offs[:, :], offs_in.ap())
            nc.sync.dma_start(iota[:, :, :], iota_in.ap())
            nc.sync.dma_start(idf[:, :], idf_in.ap())
            nc.sync.dma_start(lng[:, :, :], lng_in.ap())
            nc.sync.dma_start(lnb[:, :, :], lnb_in.ap())
            nc.sync.dma_start(Wht[:, :, :], W_in.ap().rearrange("r l k m -> k (r l) m"))
            nc.sync.dma_start(attr[:, :, :], att_in.ap().rearrange("r l p m -> p (r l) m"))
            nc.sync.dma_start(Wo[:, :, :], Wo_in.ap().rearrange("l k m -> k l m"))

            def node_scores_block(pool, psum, lhsT, Wslice, att_ap, out_ap):
                """tanh(x@W) . att summed over d -> out_ap [128,2] (f32)."""
                pt = psum.tile([128, HD], F32, tag="ns_ps")
                nc.tensor.matmul(pt[:, :], lhsT, Wslice, start=True, stop=True)
                tt = pool.tile([128, HD], F32, tag="ns_tt")
                nc.scalar.activation(tt[:, :], pt[:, :], AF.Tanh)
                tm = pool.tile([128, HD], F32, tag="ns_tm")
                nc.vector.tensor_tensor(tm[:, :], tt[:, :], att_ap, OP.mult)
                nc.vector.tensor_reduce(out_ap, tm.rearrange("p (h d) -> p h d", h=2),
                                        mybir.AxisListType.X, OP.add)

            for l in range(n_layers):
                # ================= node phase =================
                with tc.tile_pool(name="nodep", bufs=2) as np_pool, \
                     tc.tile_pool(name="nodebig", bufs=1) as np_big, \
                     tc.tile_pool(name="nodeps", bufs=2, space="PSUM") as np_psum:
                    # layernorm
                    x = ent
                    mu = np_pool.tile([128, NBLK], F32, tag="mu")
                    nc.vector.tensor_reduce(mu[:, :], x[:, :, :], mybir.AxisListType.X, OP.add)
                    nc.vector.tensor_scalar(mu[:, :], mu[:, :], 1.0 / DIM, None, OP.mult)
                    xc = np_big.tile([128, NBLK, DIM], F32, tag="xc")
                    nc.vector.tensor_tensor(
                        xc[:, :, :], x[:, :, :],
                        mu.unsqueeze(2).broadcast_to([128, NBLK, DIM]), OP.subtract)
                    sq = np_big.tile([128, NBLK, DIM], F32, tag="sq")
                    nc.vector.tensor_tensor(sq[:, :, :], xc[:, :, :], xc[:, :, :], OP.mult)
                    var = np_pool.tile([128, NBLK], F32, tag="var")
                    nc.vector.tensor_reduce(var[:, :], sq[:, :, :], mybir.AxisListType.X, OP.add)
                    nc.vector.tensor_scalar(var[:, :], var[:, :], 1.0 / DIM, LN_EPS, OP.mult, OP.add)
                    std = np_pool.tile([128, NBLK], F32, tag="std")
                    nc.scalar.activation(std[:, :], var[:, :], AF.Sqrt)
                    rstd = np_pool.tile([128, NBLK], F32, tag="rstd")
                    nc.vector.reciprocal(rstd[:, :], std[:, :])
                    nc.vector.tensor_tensor(
                        h_t[:, :, :], xc[:, :, :],
                        rstd.unsqueeze(2).broadcast_to([128, NBLK, DIM]), OP.mult)
                    nc.vector.tensor_tensor(
                        h_t[:, :, :], h_t[:, :, :],
                        lng[:, l, :].unsqueeze(1).broadcast_to([128, NBLK, DIM]),
                        OP.mult)
                    nc.vector.tensor_tensor(
                        h_t[:, :, :], h_t[:, :, :],
                        lnb[:, l, :].unsqueeze(1).broadcast_to([128, NBLK, DIM]),
                        OP.add)
                    if debug and l == 0:
                        nc.sync.dma_start(dbg_h.ap().rearrange("(b p) f -> p b f", p=128),
                                          h_t[:, :, :])

                    # transpose h -> ht [64, b, 128]
                    ht = np_big.tile([64, NBLK, 128], F32, tag="ht")
                    for b in range(NBLK):
                        ps = np_psum.tile([64, 128], F32, tag="trh")
                        nc.tensor.transpose(ps[:, :], h_t[:, b, :], idf[:, :])
                        nc.scalar.activation(ht[:, b, :], ps[:, :], AF.Copy)

                    # s_h, s_t  [128, NBLK, 2] f32
                    s_f0 = np_pool.tile([128, NBLK, 2], F32, tag="s_f0")
                    s_f1 = np_pool.tile([128, NBLK, 2], F32, tag="s_f1")
                    s_f = [s_f0, s_f1]
                    for r in range(2):
                        for b in range(NBLK):
                            node_scores_block(np_pool, np_psum, ht[:, b, :],
                                              Wht[:, r * L_LAYERS + l, :],
                                              attr[:, r * L_LAYERS + l, :],
                                              s_f[r][:, b, :])
                    # sh table rows: [bf16(s_h)(2), bf16(residual)(2), 0...]
                    shsb = np_big.tile([128, NBLK, 128], BF16, tag="shsb")
                    shm_f = np_pool.tile([128, NBLK, 2], F32, tag="shm_f")
                    nc.vector.memset(shsb[:, :, 4:128], 0.0)
                    nc.vector.tensor_copy(shsb[:, :, 0:2], s_f[0][:, :, :])
                    nc.vector.tensor_copy(shm_f[:, :, :], shsb[:, :, 0:2])
                    nc.vector.tensor_tensor(shsb[:, :, 2:4], s_f[0][:, :, :], shm_f[:, :, :],
                                            OP.subtract)
                    nc.sync.dma_start(shtab.rearrange("(b p) c -> p b c", p=128),
                                      shsb[:, :, :])

                    # gather table slice: [h | s_t | s_t_res | 1 | 0...]
                    tabsb = np_big.tile([128, NBLK, 128], BF16, tag="tabsb")
                    nc.vector.tensor_copy(tabsb[:, :, 0:64], h_t[:, :, :])
                    nc.vector.tensor_copy(tabsb[:, :, 64:66], s_f[1][:, :, :])
                    stm_f = np_pool.tile([128, NBLK, 2], F32, tag="stm_f")
                    nc.vector.tensor_copy(stm_f[:, :, :], tabsb[:, :, 64:66])
                    nc.vector.tensor_tensor(tabsb[:, :, 66:68], s_f[1][:, :, :], stm_f[:, :, :],
                                            OP.subtract)
                    nc.vector.memset(tabsb[:, :, 68:69], 1.0)
                    nc.vector.memset(tabsb[:, :, 69:128], 0.0)
                    if do_node_ag:
                        nc.sync.dma_start(tab_in.rearrange("(b p) c -> p b c", p=128),
                                          tabsb[:, :, :])
                        nc.gpsimd.collective_compute(
                            "AllGather", OP.bypass,
                            replica_groups=[list(range(CORES))],
                            ins=[tab_in.opt()], outs=[table.opt()])
                    else:
                        nc.sync.dma_start(
                            table[0:M_PER_CORE, :].rearrange("(b p) c -> p b c", p=128),
                            tabsb[:, :, :])

                    # s_r table (once, both layers)
                    if l == 0:
                        relsb = np_pool.tile([128, 4, DIM], F32, tag="relsb")
                        nc.sync.dma_start(relsb[:, :, :],
                                          rel_in.ap().rearrange("(b p) f -> p b f", p=128))
                        relt = np_pool.tile([64, 4, 128], F32, tag="relt")
                        for b in range(4):
                            ps = np_psum.tile([64, 128], F32, tag="trh")
                            nc.tensor.transpose(ps[:, :], relsb[:, b, :], idf[:, :])
                            nc.scalar.activation(relt[:, b, :], ps[:, :], AF.Copy)
                        srsb = np_pool.tile([128, 4, 128], BF16, tag="srsb")
                        nc.vector.memset(srsb[:, :, :], 0.0)
                        sr_f = np_pool.tile([128, 4, 2 * L_LAYERS], F32, tag="sr_f")
                        for ll in range(L_LAYERS):
                            for b in range(4):
                                node_scores_block(np_pool, np_psum, relt[:, b, :],
                                                  Wht[:, 2 * L_LAYERS + ll, :],
                                                  attr[:, 2 * L_LAYERS + ll, :],
                                                  sr_f[:, b, 2 * ll:2 * ll + 2])
                        nc.vector.tensor_copy(srsb[:, :, 0:4], sr_f[:, :, :])
                        srm_f = np_pool.tile([128, 4, 4], F32, tag="srm_f")
                        nc.vector.tensor_copy(srm_f[:, :, :], srsb[:, :, 0:4])
                        nc.vector.tensor_tensor(srsb[:, :, 4:8], sr_f[:, :, :], srm_f[:, :, :],
                                                OP.subtract)
                        nc.sync.dma_start(srtab.rearrange("(b p) c -> p b c", p=128),
                                          srsb[:, :, :])

                # ================= edge phase =================
                for it in range(n_iters):
                    with tc.tile_pool(name="edgep", bufs=2) as ep, \
                         tc.tile_pool(name="edgeps", bufs=2) as eps, \
                         tc.tile_pool(name="edgep2", bufs=2) as ep2, \
                         tc.tile_pool(name="msgp", bufs=1) as msgp, \
                         tc.tile_pool(name="spmm_ps", bufs=2, space="PSUM") as spmm_ps, \
                         tc.tile_pool(name="zps_pool", bufs=2, space="PSUM") as zps_pool:
                        for k in range(NPIECE):
                            psb = spmm_ps.tile([128, BPP, 128], F32, tag="blkps")
                            psz = zps_pool.tile([128, BPP, 2], F32, tag="zps")
                            for p in range(2):
                                slot0 = p * C_PASS + k * PC
                                Gt = ep.tile([128, PC, 128], BF16, tag="Gt")
                                nc.gpsimd.dma_gather(
                                    out_ap=Gt[:, :, :],
                                    in_ap=table[p * SPLIT:, :],
                                    idxs_ap=gidx[:, p, k * (NP_IDX // 16):(k + 1) * (NP_IDX // 16)],
                                    num_idxs=NP_IDX, num_idxs_reg=NP_IDX, elem_size=128, single_packet=False)
                                # one-hot [128e, 128n, c] built on DVE; chunk dim
                                # innermost so every operand is packed (2x mode)
                                oh = ep2.tile([128, 128, PC], BF16, tag="oh")
                                nc.vector.tensor_tensor(
                                    oh[:, :, :],
                                    offs[:, slot0:slot0 + PC].unsqueeze(1)
                                        .broadcast_to([128, 128, PC]),
                                    iota[:, :, :],
                                    OP.is_equal)

                                if it == 0:
                                    idx0 = (p * NPIECE + k) * (NP_IDX // 16)
                                    idx1 = (p * NPIECE + k + 1) * (NP_IDX // 16)
                                    if l == 0:
                                        SRt = eps.tile([128, PC, 128], BF16, tag="SRt")
                                        nc.gpsimd.dma_gather(
                                            out_ap=SRt[:, :, :],
                                            in_ap=srtab[:, :],
                                            idxs_ap=tidx[:, idx0:idx1],
                                            num_idxs=NP_IDX, num_idxs_reg=NP_IDX, elem_size=128, single_packet=False)
                                        nc.vector.tensor_copy(
                                            SRx[:, slot0:slot0 + PC, :], SRt[:, :, 0:8])
                                    SHt = eps.tile([128, PC, 128], BF16, tag="SHt")
                                    nc.gpsimd.dma_gather(
                                        out_ap=SHt[:, :, :],
                                        in_ap=shtab[:, :],
                                        idxs_ap=shidx[:, idx0:idx1],
                                        num_idxs=NP_IDX, num_idxs_reg=NP_IDX, elem_size=128, single_packet=False)
                                    # scores
                                    sc = ep2.tile([128, PC, 2], F32, tag="sc")
                                    nc.vector.tensor_tensor(sc[:, :, :], Gt[:, :, 64:66],
                                                            Gt[:, :, 66:68], OP.add)
                                    t2 = ep2.tile([128, PC, 2], F32, tag="t2")
                                    nc.vector.tensor_tensor(
                                        t2[:, :, :],
                                        SRx[:, slot0:slot0 + PC, 2 * l:2 * l + 2],
                                        SRx[:, slot0:slot0 + PC, 4 + 2 * l:6 + 2 * l], OP.add)
                                    nc.vector.tensor_tensor(sc[:, :, :], sc[:, :, :], t2[:, :, :],
                                                            OP.add)
                                    nc.vector.tensor_tensor(t2[:, :, :], SHt[:, :, 0:2],
                                                            SHt[:, :, 2:4], OP.add)
                                    nc.vector.tensor_tensor(sc[:, :, :], sc[:, :, :], t2[:, :, :],
                                                            OP.add)
                                    nc.vector.scalar_tensor_tensor(
                                        sc[:, :, :], sc[:, :, :], LRELU, sc[:, :, :],
                                        OP.mult, OP.max)
                                    nc.scalar.activation(At[:, slot0:slot0 + PC, :],
                                                         sc[:, :, :], AF.Exp)

                                # messages: Z table is (d, h)-interleaved so At rides
                                # as a packed innermost-2 operand (2x DVE mode)
                                if it == 0:
                                    # it0 rows are plain [h(64)|s_t|res|1|..]: write
                                    # interleaved msg[(d,h)] = h[d]*At[h]
                                    msg = msgp.tile([128, PC, 128], BF16, tag="msg")
                                    mv = msg.rearrange("p c (d o) -> p c d o", o=2)
                                    for o in range(2):
                                        nc.vector.tensor_tensor(
                                            mv[:, :, :, o],
                                            Gt[:, :, 0:64],
                                            At[:, slot0:slot0 + PC, o].unsqueeze(2)
                                                .broadcast_to([128, PC, 64]),
                                            OP.mult)
                                else:
                                    msg = Gt
                                    gv = Gt.rearrange("p c (d o) -> p c d o", o=2)
                                    nc.vector.tensor_tensor(
                                        gv[:, :, :, :], gv[:, :, :, :],
                                        At[:, slot0:slot0 + PC, :].unsqueeze(2)
                                            .broadcast_to([128, PC, 64, 2]),
                                        OP.mult)
                                # spmm (+ z columns via separate 2-col matmul on iter 0)
                                # PSUM zero-regions are whole banks: start/stop once per bank.
                                for j in range(PC):
                                    bl = BLKMAP[j]
                                    st = (p == 0) and (j == 0 or j == J_B1)
                                    sp = (p == 1) and (j == J_B1 - 1 or j == PC - 1)
                                    nc.tensor.matmul(
                                        psb[:, bl, :], oh[:, :, j], msg[:, j, :],
                                        start=st, stop=sp)
                                    if it == 0:
                                        nc.tensor.matmul(
                                            psz[:, bl, :], oh[:, :, j],
                                            At[:, slot0 + j, :],
                                            start=(p == 0 and j == 0),
                                            stop=(p == 1 and j == PC - 1))

                            # piece epilogue: z, recip, Z assembly
                            if it == 0:
                                b0 = k * BPP
                                nc.vector.tensor_scalar(zt[:, b0:b0 + BPP, :], psz[:, :, :],
                                                        1e-30, None, OP.max)
                                nc.vector.reciprocal(recip[:, b0:b0 + BPP, :],
                                                     zt[:, b0:b0 + BPP, :])
                                nc.vector.tensor_scalar(recip[:, b0:b0 + BPP, :],
                                                        recip[:, b0:b0 + BPP, :],
                                                        1.0 - ALPHA, None, OP.mult)
                            zv = Z.rearrange("p b (d o) -> p b d o", o=2)
                            pv = psb.rearrange("p b (d o) -> p b d o", o=2)
                            for bl in range(BPP):
                                b = k * BPP + bl
                                nc.scalar.activation(zv[:, b, :, 0], pv[:, bl, :, 0], AF.Copy,
                                                     scale=recip[:, b, 0:1])
                                nc.scalar.activation(zv[:, b, :, 1], pv[:, bl, :, 1],
                                                     AF.Copy, scale=recip[:, b, 1:2])
                                nc.vector.scalar_tensor_tensor(
                                    zv[:, b, :, :],
                                    h_t[:, b, :].unsqueeze(2)
                                        .broadcast_to([128, 64, 2]),
                                    ALPHA,
                                    zv[:, b, :, :],
                                    OP.mult, OP.add)

                        if debug and l == 0 and it == 0:
                            nc.sync.dma_start(dbg_z0.ap().rearrange("(b p) c -> p b c", p=128),
                                              Z[:, :, :])
                            nc.sync.dma_start(
                                dbg_at.ap(),
                                At[:, :, :])

                        if it < n_iters - 1:
                            tabz = ep.tile([128, NBLK, 128], BF16, tag="Gt")
                            nc.vector.tensor_copy(tabz[:, :, :], Z[:, :, :])
                            if do_ag:
                                nc.sync.dma_start(tab_in.rearrange("(b p) c -> p b c", p=128),
                                                  tabz[:, :, :])
                                nc.gpsimd.collective_compute(
                                    "AllGather", OP.bypass,
                                    replica_groups=[list(range(CORES))],
                                    ins=[tab_in.opt()], outs=[table.opt()])
                            else:
                                nc.sync.dma_start(
                                    table[0:M_PER_CORE, :].rearrange("(b p) c -> p b c", p=128),
                                    tabz[:, :, :])

                # ================= conv + residual =================
                with tc.tile_pool(name="convp", bufs=2) as cp, \
                     tc.tile_pool(name="convps", bufs=4, space="PSUM") as cps:
                    for b in range(NBLK):
                        pzt = cps.tile([128, 128], F32, tag="pzt")
                        nc.tensor.transpose(pzt[:, :], Z[:, b, :], idf[:, :])
                        Zt = cp.tile([128, 128], F32, tag="Zt")
                        nc.scalar.activation(Zt[:, :], pzt[:, :], AF.Copy)
                        pc_ = cps.tile([128, 64], F32, tag="pc")
                        nc.tensor.matmul(pc_[:, :], Zt[:, :], Wo[:, l, :],
                                         start=True, stop=True)
                        nc.vector.tensor_tensor(ent[:, b, :], pc_[:, :], ent[:, b, :], OP.add)

            nc.sync.dma_start(out_ext.ap().rearrange("(b p) f -> p b f", p=128),
                              ent[:, :, :])

    nc.compile()
    return nc


# ----------------------------------------------------------------------------
# host wrapper
# ----------------------------------------------------------------------------

def _make_in_maps(inputs, plans):
    ent = np.asarray(inputs["entity_embed"], np.float32)
    rel = np.zeros((512, DIM), np.float32)
    rel[:N_REL] = np.asarray(inputs["relation_embed"], np.float32)
    lng = np.tile(np.asarray(inputs["ln_gamma"], np.float32)[None], (128, 1, 1))
    lnb = np.tile(np.asarray(inputs["ln_beta"], np.float32)[None], (128, 1, 1))
    W = np.stack([np.asarray(inputs["W_h"], np.float32),
                  np.asarray(inputs["W_t"], np.float32),
                  np.asarray(inputs["W_r"], np.float32)], axis=0)
    att = np.stack([np.asarray(inputs["att_h"], np.float32),
                    np.asarray(inputs["att_t"], np.float32),
                    np.asarray(inputs["att_r"], np.float32)], axis=0)
    att_rep = np.tile(att.reshape(3, L_LAYERS, 1, HD), (1, 1, 128, 1)).astype(np.float32)
    Wo = np.asarray(inputs["W_o"], np.float32)
    # Z columns are (d, h)-interleaved on device; permute W_o rows to match
    il = np.arange(HD)
    Wo = np.ascontiguousarray(Wo[:, (il % 2) * DIM + il // 2, :])
    iota = np.tile(np.arange(128, dtype=np.float32)[:, None], (1, PC))
    iota = np.tile(iota[None], (128, 1, 1)).astype(BF)
    idf = np.eye(128, dtype=np.float32)

    common = dict(rel_emb=rel, ln_g=np.ascontiguousarray(lng), ln_b=np.ascontiguousarray(lnb),
                  W_htr=np.ascontiguousarray(W), att_rep=np.ascontiguousarray(att_rep),
                  W_o=Wo, iota_exp=np.ascontiguousarray(iota), ident_f=idf)
    in_maps = []
    for pl in plans:
        sl = np.zeros((M_PER_CORE, DIM), np.float32)
        nv = pl["n_valid"]
        sl[pl["perm"][:nv]] = ent[pl["node_lo"]:pl["node_lo"] + nv]
        m = dict(common)
        m["ent_slice"] = sl
        m["gidx"] = np.ascontiguousarray(np.stack(pl["gidx"], axis=1))
        m["tidx"] = pl["tidx"]
        m["shidx"] = pl["shidx"]
        m["offs"] = pl["offs"]
        in_maps.append(m)
    return in_maps


_CACHE = {}


def _get_nc(meta_key, meta, debug):
    key = (meta_key, debug)
    if key not in _CACHE:
        _CACHE[key] = _build_nc(meta, debug=debug)
    return _CACHE[key]


def run(inputs, debug=False, trace=False):
    plans, meta = _build_plan(np.asarray(inputs["edge_index"]),
                              np.asarray(inputs["edge_type"]))
    nc = _get_nc((meta["C_PASS"],), meta, debug)
    in_maps = _make_in_maps(inputs, plans)
    res = bass_utils.run_bass_kernel_spmd(nc, in_maps, core_ids=list(range(CORES)),
                                          trace=trace)
    out = np.zeros((N_ENT, DIM), np.float32)
    for c, pl in enumerate(plans):
        nv = pl["n_valid"]
        sl = np.asarray(res.results[c]["out_slice"])
        out[pl["node_lo"]:pl["node_lo"] + nv] = sl[pl["perm"][:nv]]
    return out, res, plans, meta


def kernel(**inputs) -> np.ndarray:
    out, _, _, _ = run(inputs)
    return out.astype(np.asarray(inputs["entity_embed"]).dtype)


# revision 18
# speedup vs baseline: 1.1833x; 1.0021x over previous
"""Trainium2 Bass kernel for nn_DAGNLinkPrediction (GNN message passing).

Self-contained: host-side integer preprocessing (sharding/permutation) + bass/tile
kernel + SPMD launch across 8 NeuronCores via run_bass_kernel_spmd.

Sharding: edges partitioned by src owner core (6272 rows/core). Within each core,
nodes are PERMUTED into 49 blocks of 128 rows so that per-(dst-half, block) edge
counts fit a fixed chunk-capacity pattern (6,5,5,5,5,5,5 per 7-block piece) —
this cuts edge-chunk padding from ~40% to ~2.5%. Per power iteration each core
gathers Z[dst] rows (256B bf16) with gpsimd.dma_gather, computes messages,
segment-sums by src via PE matmuls with 0/1 one-hot matrices, and AllGathers the
updated bf16 node table.
"""
import numpy as np
import ml_dtypes

from concourse import bass, bacc, tile, bass_utils, mybir

BF = ml_dtypes.bfloat16
F32 = mybir.dt.float32
BF16 = mybir.dt.bfloat16
I16 = mybir.dt.int16

CORES = 8
N_ENT = 50000
N_REL = 500
HEADS = 2
DIM = 64
HD = HEADS * DIM                 # 128
M_PER_CORE = 6272                # 49*128
NBLK = M_PER_CORE // 128         # 49
TAB_ROWS = CORES * M_PER_CORE    # 50176
SPLIT = TAB_ROWS // 2            # 25088 == 4*M_PER_CORE (core boundary)
NPIECE = 7                       # pieces per pass; NBLK = 7*7
BPP = NBLK // NPIECE             # blocks per piece = 7
ALPHA = 0.15
LN_EPS = 1e-5
L_LAYERS = 2
POW_ITER = 3
LRELU = 0.01

# chunk capacities per block within a piece (sums to PC chunks per piece)
CAP_PAT = (6, 5, 5, 5, 5, 5, 5)
CUM = (0, 6, 11, 16, 21, 26, 31)          # chunk offset of block-in-piece
PC = sum(CAP_PAT)                          # 36 chunks per piece
C_PASS = NPIECE * PC                       # 252 chunks per half
NIDX = C_PASS * 128                        # idx slots per half
NIDX_PIECE = PC * 128
J_B1 = CUM[4]                              # first chunk of PSUM bank 1
BLKMAP = tuple(i for i in range(BPP) for _ in range(CAP_PAT[i]))

AF = mybir.ActivationFunctionType
OP = mybir.AluOpType


# ----------------------------------------------------------------------------
# host-side preprocessing (integer/layout only)
# ----------------------------------------------------------------------------

def _wrap_idxs(idx):
    n = idx.shape[0]
    base = idx.reshape(n // 16, 16).T.astype(np.int16)
    return np.ascontiguousarray(np.tile(base, (8, 1)))


def _balance_core(deg):
    """Assign 6272 local nodes to 49 blocks s.t. per-(half, block) edge counts
    fit cap[b]*128. Returns perm: local node id -> local row."""
    caps = np.array([CAP_PAT[b % BPP] for b in range(NBLK)], np.int64)
    capn = caps * 128
    tot = deg.sum(1)
    order = np.argsort(-tot, kind="stable")
    loads = np.zeros((NBLK, 2), np.int64)
    fill = np.zeros(NBLK, np.int64)
    blk_of = np.full(M_PER_CORE, -1, np.int64)
    for n in order:
        dlo, dhi = deg[n]
        if dlo == 0 and dhi == 0:
            break
        feas = (fill < 128) & (loads[:, 0] + dlo <= capn) & (loads[:, 1] + dhi <= capn)
        if not feas.any():
            raise RuntimeError("balance infeasible")
        u = np.maximum((loads[:, 0] + dlo) / capn, (loads[:, 1] + dhi) / capn)
        u[~feas] = np.inf
        b = int(np.argmin(u))
        loads[b, 0] += dlo
        loads[b, 1] += dhi
        fill[b] += 1
        blk_of[n] = b
    # zero-degree nodes fill remaining slots
    rem = np.where(blk_of < 0)[0]
    space = np.repeat(np.arange(NBLK), (128 - fill).astype(np.int64))
    blk_of[rem] = space[: len(rem)]
    # rows within block: placement order
    perm = np.zeros(M_PER_CORE, np.int64)
    pos = np.zeros(NBLK, np.int64)
    for n in np.concatenate([order[blk_of[order] >= 0][: (tot > 0).sum()], rem]):
        b = blk_of[n]
        perm[n] = b * 128 + pos[b]
        pos[b] += 1
    assert (pos == 128).all()
    return perm


def _build_plan(edge_index, edge_type):
    src = edge_index[0].astype(np.int64)
    dst = edge_index[1].astype(np.int64)
    et = edge_type.astype(np.int64)

    half = ((dst // M_PER_CORE) >= (CORES // 2)).astype(np.int64)
    deg = np.zeros((CORES * M_PER_CORE, 2), np.int64)
    np.add.at(deg, (src, half), 1)

    perms = []
    row_global = np.zeros(CORES * M_PER_CORE, np.int64)
    for c in range(CORES):
        p = _balance_core(deg[c * M_PER_CORE:(c + 1) * M_PER_CORE])
        perms.append(p)
        row_global[c * M_PER_CORE:(c + 1) * M_PER_CORE] = c * M_PER_CORE + p

    src_row = row_global[src]          # permuted local+core row of src
    dst_row = row_global[dst]          # permuted global row of dst
    core_of = src // M_PER_CORE
    blk = (src_row % M_PER_CORE) // 128

    slot_start = np.array([(b // BPP) * PC + CUM[b % BPP] for b in range(NBLK)],
                          np.int64)

    key = (core_of * 2 + half) * NBLK + blk
    order = np.argsort(key, kind="stable")
    key_sorted = key[order]
    bounds = np.searchsorted(key_sorted, np.arange(CORES * 2 * NBLK + 1))

    plans = []
    for c in range(CORES):
        pl = {}
        pl["node_lo"] = c * M_PER_CORE
        pl["n_valid"] = max(0, min(N_ENT - c * M_PER_CORE, M_PER_CORE))
        pl["perm"] = perms[c]
        gidx, offs, typs, shls = [], [], [], []
        for p in range(2):
            eids = np.full(NIDX, -1, np.int64)
            for b in range(NBLK):
                k = (c * 2 + p) * NBLK + b
                lst = order[bounds[k]:bounds[k + 1]]
                assert len(lst) <= CAP_PAT[b % BPP] * 128
                s0 = slot_start[b] * 128
                eids[s0:s0 + len(lst)] = lst
            pad = eids < 0
            e_safe = np.where(pad, 0, eids)
            didx = dst_row[e_safe] - p * SPLIT
            didx[pad] = 0
            off = (src_row[e_safe] % 128).astype(np.int64)
            off[pad] = -1
            shl = (src_row[e_safe] % M_PER_CORE).astype(np.int64)
            shl[pad] = 0
            t = et[e_safe].copy()
            t[pad] = 0
            gidx.append(_wrap_idxs(didx.astype(np.int16)))
            offs.append(off.reshape(C_PASS, 128).T.astype(np.float32))  # [128, C_PASS]
            typs.append(t)
            shls.append(shl)
        pl["gidx"] = gidx
        pl["tidx"] = _wrap_idxs(np.concatenate(typs).astype(np.int16))
        pl["shidx"] = _wrap_idxs(np.concatenate(shls).astype(np.int16))
        pl["offs"] = np.concatenate(offs, axis=1).astype(BF)  # [128, 2*C_PASS]
        plans.append(pl)

    meta = dict(C_PASS=C_PASS, NIDX=NIDX, PIECE_CHUNKS=PC, NIDX_PIECE=NIDX_PIECE)
    return plans, meta


# ----------------------------------------------------------------------------
# device kernel
# ----------------------------------------------------------------------------

def _build_nc(meta, debug=False, n_layers=L_LAYERS, n_iters=POW_ITER, do_ag=True, do_node_ag=True):
    NP_IDX = meta["NIDX_PIECE"]        # idxs per piece
    IW = meta["NIDX"] // 16            # idx cols per pass

    nc = bacc.Bacc("TRN2", target_bir_lowering=False, debug=False,
                   num_devices=CORES)

    # ---- external inputs ----
    ent_in = nc.dram_tensor("ent_slice", [M_PER_CORE, DIM], F32, kind="ExternalInput")
    rel_in = nc.dram_tensor("rel_emb", [512, DIM], F32, kind="ExternalInput")
    lng_in = nc.dram_tensor("ln_g", [128, L_LAYERS, DIM], F32, kind="ExternalInput")
    lnb_in = nc.dram_tensor("ln_b", [128, L_LAYERS, DIM], F32, kind="ExternalInput")
    W_in = nc.dram_tensor("W_htr", [3, L_LAYERS, DIM, HD], F32, kind="ExternalInput")
    att_in = nc.dram_tensor("att_rep", [3, L_LAYERS, 128, HD], F32, kind="ExternalInput")
    Wo_in = nc.dram_tensor("W_o", [L_LAYERS, HD, DIM], F32, kind="ExternalInput")
    gidx_in = nc.dram_tensor("gidx", [128, 2, IW], I16, kind="ExternalInput")
    tidx_in = nc.dram_tensor("tidx", [128, 2 * IW], I16, kind="ExternalInput")
    shidx_in = nc.dram_tensor("shidx", [128, 2 * IW], I16, kind="ExternalInput")
    offs_in = nc.dram_tensor("offs", [128, 2 * C_PASS], BF16, kind="ExternalInput")
    iota_in = nc.dram_tensor("iota_exp", [128, 128, PC], BF16, kind="ExternalInput")
    idf_in = nc.dram_tensor("ident_f", [128, 128], F32, kind="ExternalInput")

    out_ext = nc.dram_tensor("out_slice", [M_PER_CORE, DIM], F32, kind="ExternalOutput")
    if debug:
        dbg_at = nc.dram_tensor("dbg_at", [128, 2 * C_PASS, 2], BF16, kind="ExternalOutput")
        dbg_z0 = nc.dram_tensor("dbg_z0", [M_PER_CORE, HD], F32, kind="ExternalOutput")
        dbg_h = nc.dram_tensor("dbg_h", [M_PER_CORE, DIM], F32, kind="ExternalOutput")

    with tile.TileContext(nc) as tc:
        with tc.tile_pool(name="dram", bufs=1, space="DRAM") as dram, \
             tc.tile_pool(name="persist", bufs=1) as pp:
            table = dram.tile([TAB_ROWS, 128], BF16, tag="table")
            tab_in = dram.tile([M_PER_CORE, 128], BF16, tag="tab_in")
            srtab = dram.tile([512, 128], BF16, tag="srtab")
            shtab = dram.tile([M_PER_CORE, 128], BF16, tag="shtab")

            ent = pp.tile([128, NBLK, DIM], F32, tag="ent")
            h_t = pp.tile([128, NBLK, DIM], F32, tag="h")
            recip = pp.tile([128, NBLK, 2], F32, tag="recip")
            zt = pp.tile([128, NBLK, 2], F32, tag="zt")
            At = pp.tile([128, 2 * C_PASS, 2], BF16, tag="At")
            SRx = pp.tile([128, 2 * C_PASS, 8], BF16, tag="SRx")
            Z = pp.tile([128, NBLK, HD], F32, tag="Z")
            gidx = pp.tile([128, 2, IW], I16, tag="gidx")
            tidx = pp.tile([128, 2 * IW], I16, tag="tidx")
            shidx = pp.tile([128, 2 * IW], I16, tag="shidx")
            offs = pp.tile([128, 2 * C_PASS], BF16, tag="offs")
            iota = pp.tile([128, 128, PC], BF16, tag="iota")
            idf = pp.tile([128, 128], F32, tag="idf")
            lng = pp.tile([128, L_LAYERS, DIM], F32, tag="lng")
            lnb = pp.tile([128, L_LAYERS, DIM], F32, tag="lnb")
            Wht = pp.tile([64, 3 * L_LAYERS, HD], F32, tag="Wht")
            attr = pp.tile([128, 3 * L_LAYERS, HD], F32, tag="attr")
            Wo = pp.tile([HD, L_LAYERS, DIM], F32, tag="Wo")

            # ---- load inputs ----
            nc.sync.dma_start(ent[:, :, :], ent_in.ap().rearrange("(b p) f -> p b f", p=128))
            nc.sync.dma_start(gidx[:, :, :], gidx_in.ap())
            nc.sync.dma_start(tidx[:, :], tidx_in.ap())
            nc.sync.dma_start(shidx[:, :], shidx_in.ap())
            nc.sync.dma_start(offs[:, :], offs_in.ap())
            nc.sync.dma_start(iota[:, :, :], iota_in.ap())
            nc.sync.dma_start(idf[:, :], idf_in.ap())
            nc.sync.dma_start(lng[:, :, :], lng_in.ap())
            nc.sync.dma_start(lnb[:, :, :], lnb_in.ap())
            nc.sync.dma_start(Wht[:, :, :], W_in.ap().rearrange("r l k m -> k (r l) m"))
            nc.sync.dma_start(attr[:, :, :], att_in.ap().rearrange("r l p m -> p (r l) m"))
            nc.sync.dma_start(Wo[:, :, :], Wo_in.ap().rearrange("l k m -> k l m"))

            def node_scores_block(pool, psum, lhsT, Wslice, att_ap, out_ap):
                """tanh(x@W) . att summed over d -> out_ap [128,2] (f32)."""
                pt = psum.tile([128, HD], F32, tag="ns_ps")
                nc.tensor.matmul(pt[:, :], lhsT, Wslice, start=True, stop=True)
                tt = pool.tile([128, HD], F32, tag="ns_tt")
                nc.scalar.activation(tt[:, :], pt[:, :], AF.Tanh)
                tm = pool.tile([128, HD], F32, tag="ns_tm")
                nc.vector.tensor_tensor(tm[:, :], tt[:, :], att_ap, OP.mult)
                nc.vector.tensor_reduce(out_ap, tm.rearrange("p (h d) -> p h d", h=2),
                                        mybir.AxisListType.X, OP.add)

            for l in range(n_layers):
                # ================= node phase =================
                with tc.tile_pool(name="nodep", bufs=2) as np_pool, \
                     tc.tile_pool(name="nodebig", bufs=1) as np_big, \
                     tc.tile_pool(name="nodeps", bufs=2, space="PSUM") as np_psum:
                    # layernorm
                    x = ent
                    mu = np_pool.tile([128, NBLK], F32, tag="mu")
                    nc.vector.tensor_reduce(mu[:, :], x[:, :, :], mybir.AxisListType.X, OP.add)
                    nc.vector.tensor_scalar(mu[:, :], mu[:, :], 1.0 / DIM, None, OP.mult)
                    xc = np_big.tile([128, NBLK, DIM], F32, tag="xc")
                    nc.vector.tensor_tensor(
                        xc[:, :, :], x[:, :, :],
                        mu.unsqueeze(2).broadcast_to([128, NBLK, DIM]), OP.subtract)
                    sq = np_big.tile([128, NBLK, DIM], F32, tag="sq")
                    nc.vector.tensor_tensor(sq[:, :, :], xc[:, :, :], xc[:, :, :], OP.mult)
                    var = np_pool.tile([128, NBLK], F32, tag="var")
                    nc.vector.tensor_reduce(var[:, :], sq[:, :, :], mybir.AxisListType.X, OP.add)
                    nc.vector.tensor_scalar(var[:, :], var[:, :], 1.0 / DIM, LN_EPS, OP.mult, OP.add)
                    std = np_pool.tile([128, NBLK], F32, tag="std")
                    nc.scalar.activation(std[:, :], var[:, :], AF.Sqrt)
                    rstd = np_pool.tile([128, NBLK], F32, tag="rstd")
                    nc.vector.reciprocal(rstd[:, :], std[:, :])
                    nc.vector.tensor_tensor(
                        h_t[:, :, :], xc[:, :, :],
                        rstd.unsqueeze(2).broadcast_to([128, NBLK, DIM]), OP.mult)
                    nc.vector.tensor_tensor(
                        h_t[:, :, :], h_t[:, :, :],
                        lng[:, l, :].unsqueeze(1).broadcast_to([128, NBLK, DIM]),
                        OP.mult)
                    nc.vector.tensor_tensor(
                        h_t[:, :, :], h_t[:, :, :],
                        lnb[:, l, :].unsqueeze(1).broadcast_to([128, NBLK, DIM]),
                        OP.add)
                    if debug and l == 0:
                        nc.sync.dma_start(dbg_h.ap().rearrange("(b p) f -> p b f", p=128),
                                          h_t[:, :, :])

                    # transpose h -> ht [64, b, 128]
                    ht = np_big.tile([64, NBLK, 128], F32, tag="ht")
                    for b in range(NBLK):
                        ps = np_psum.tile([64, 128], F32, tag="trh")
                        nc.tensor.transpose(ps[:, :], h_t[:, b, :], idf[:, :])
                        nc.scalar.activation(ht[:, b, :], ps[:, :], AF.Copy)

                    # s_h, s_t  [128, NBLK, 2] f32
                    s_f0 = np_pool.tile([128, NBLK, 2], F32, tag="s_f0")
                    s_f1 = np_pool.tile([128, NBLK, 2], F32, tag="s_f1")
                    s_f = [s_f0, s_f1]
                    for r in range(2):
                        for b in range(NBLK):
                            node_scores_block(np_pool, np_psum, ht[:, b, :],
                                              Wht[:, r * L_LAYERS + l, :],
                                              attr[:, r * L_LAYERS + l, :],
                                              s_f[r][:, b, :])
                    # sh table rows: [bf16(s_h)(2), bf16(residual)(2), 0...]
                    shsb = np_big.tile([128, NBLK, 128], BF16, tag="shsb")
                    shm_f = np_pool.tile([128, NBLK, 2], F32, tag="shm_f")
                    nc.vector.memset(shsb[:, :, 4:128], 0.0)
                    nc.vector.tensor_copy(shsb[:, :, 0:2], s_f[0][:, :, :])
                    nc.vector.tensor_copy(shm_f[:, :, :], shsb[:, :, 0:2])
                    nc.vector.tensor_tensor(shsb[:, :, 2:4], s_f[0][:, :, :], shm_f[:, :, :],
                                            OP.subtract)
                    nc.sync.dma_start(shtab.rearrange("(b p) c -> p b c", p=128),
                                      shsb[:, :, :])

                    # gather table slice: [h | s_t | s_t_res | 1 | 0...]
                    tabsb = np_big.tile([128, NBLK, 128], BF16, tag="tabsb")
                    nc.vector.tensor_copy(tabsb[:, :, 0:64], h_t[:, :, :])
                    nc.vector.tensor_copy(tabsb[:, :, 64:66], s_f[1][:, :, :])
                    stm_f = np_pool.tile([128, NBLK, 2], F32, tag="stm_f")
                    nc.vector.tensor_copy(stm_f[:, :, :], tabsb[:, :, 64:66])
                    nc.vector.tensor_tensor(tabsb[:, :, 66:68], s_f[1][:, :, :], stm_f[:, :, :],
                                            OP.subtract)
                    nc.vector.memset(tabsb[:, :, 68:69], 1.0)
                    nc.vector.memset(tabsb[:, :, 69:128], 0.0)
                    if do_node_ag:
                        nc.sync.dma_start(tab_in.rearrange("(b p) c -> p b c", p=128),
                                          tabsb[:, :, :])
                        nc.gpsimd.collective_compute(
                            "AllGather", OP.bypass,
                            replica_groups=[list(range(CORES))],
                            ins=[tab_in.opt()], outs=[table.opt()])
                    else:
                        nc.sync.dma_start(
                            table[0:M_PER_CORE, :].rearrange("(b p) c -> p b c", p=128),
                            tabsb[:, :, :])

                    # s_r table (once, both layers)
                    if l == 0:
                        relsb = np_pool.tile([128, 4, DIM], F32, tag="relsb")
                        nc.sync.dma_start(relsb[:, :, :],
                                          rel_in.ap().rearrange("(b p) f -> p b f", p=128))
                        relt = np_pool.tile([64, 4, 128], F32, tag="relt")
                        for b in range(4):
                            ps = np_psum.tile([64, 128], F32, tag="trh")
                            nc.tensor.transpose(ps[:, :], relsb[:, b, :], idf[:, :])
                            nc.scalar.activation(relt[:, b, :], ps[:, :], AF.Copy)
                        srsb = np_pool.tile([128, 4, 128], BF16, tag="srsb")
                        nc.vector.memset(srsb[:, :, :], 0.0)
                        sr_f = np_pool.tile([128, 4, 2 * L_LAYERS], F32, tag="sr_f")
                        for ll in range(L_LAYERS):
                            for b in range(4):
                                node_scores_block(np_pool, np_psum, relt[:, b, :],
                                                  Wht[:, 2 * L_LAYERS + ll, :],
                                                  attr[:, 2 * L_LAYERS + ll, :],
                                                  sr_f[:, b, 2 * ll:2 * ll + 2])
                        nc.vector.tensor_copy(srsb[:, :, 0:4], sr_f[:, :, :])
                        srm_f = np_pool.tile([128, 4, 4], F32, tag="srm_f")
                        nc.vector.tensor_copy(srm_f[:, :, :], srsb[:, :, 0:4])
                        nc.vector.tensor_tensor(srsb[:, :, 4:8], sr_f[:, :, :], srm_f[:, :, :],
                                                OP.subtract)
                        nc.sync.dma_start(srtab.rearrange("(b p) c -> p b c", p=128),
                                          srsb[:, :, :])

                # ================= edge phase =================
                for it in range(n_iters):
                    with tc.tile_pool(name="edgep", bufs=2 if it == 0 else 3) as ep, \
                         tc.tile_pool(name="edgeps", bufs=2) as eps, \
                         tc.tile_pool(name="edgep2", bufs=2) as ep2, \
                         tc.tile_pool(name="msgp", bufs=1 if it == 0 else 2) as msgp, \
                         tc.tile_pool(name="spmm_ps", bufs=2, space="PSUM") as spmm_ps, \
                         tc.tile_pool(name="zps_pool", bufs=2, space="PSUM") as zps_pool:
                        for k in range(NPIECE):
                            psb = spmm_ps.tile([128, BPP, 128], F32, tag="blkps")
                            psz = zps_pool.tile([128, BPP, 2], F32, tag="zps")
                            for p in range(2):
                                slot0 = p * C_PASS + k * PC
                                Gt = ep.tile([128, PC, 128], BF16, tag="Gt")
                                nc.gpsimd.dma_gather(
                                    out_ap=Gt[:, :, :],
                                    in_ap=table[p * SPLIT:, :],
                                    idxs_ap=gidx[:, p, k * (NP_IDX // 16):(k + 1) * (NP_IDX // 16)],
                                    num_idxs=NP_IDX, num_idxs_reg=NP_IDX, elem_size=128, single_packet=False)
                                # one-hot [128e, 128n, c] built on DVE; chunk dim
                                # innermost so every operand is packed (2x mode)
                                oh = ep2.tile([128, 128, PC], BF16, tag="oh")
                                nc.vector.tensor_tensor(
                                    oh[:, :, :],
                                    offs[:, slot0:slot0 + PC].unsqueeze(1)
                                        .broadcast_to([128, 128, PC]),
                                    iota[:, :, :],
                                    OP.is_equal)

                                if it == 0:
                                    idx0 = (p * NPIECE + k) * (NP_IDX // 16)
                                    idx1 = (p * NPIECE + k + 1) * (NP_IDX // 16)
                                    if l == 0:
                                        SRt = eps.tile([128, PC, 128], BF16, tag="SRt")
                                        nc.gpsimd.dma_gather(
                                            out_ap=SRt[:, :, :],
                                            in_ap=srtab[:, :],
                                            idxs_ap=tidx[:, idx0:idx1],
                                            num_idxs=NP_IDX, num_idxs_reg=NP_IDX, elem_size=128, single_packet=False)
                                        nc.vector.tensor_copy(
                                            SRx[:, slot0:slot0 + PC, :], SRt[:, :, 0:8])
                                    SHt = eps.tile([128, PC, 128], BF16, tag="SHt")
                                    nc.gpsimd.dma_gather(
                                        out_ap=SHt[:, :, :],
                                        in_ap=shtab[:, :],
                                        idxs_ap=shidx[:, idx0:idx1],
                                        num_idxs=NP_IDX, num_idxs_reg=NP_IDX, elem_size=128, single_packet=False)
                                    # scores
                                    sc = ep2.tile([128, PC, 2], F32, tag="sc")
                                    nc.vector.tensor_tensor(sc[:, :, :], Gt[:, :, 64:66],
                                                            Gt[:, :, 66:68], OP.add)
                                    t2 = ep2.tile([128, PC, 2], F32, tag="t2")
                                    nc.vector.tensor_tensor(
                                        t2[:, :, :],
                                        SRx[:, slot0:slot0 + PC, 2 * l:2 * l + 2],
                                        SRx[:, slot0:slot0 + PC, 4 + 2 * l:6 + 2 * l], OP.add)
                                    nc.vector.tensor_tensor(sc[:, :, :], sc[:, :, :], t2[:, :, :],
                                                            OP.add)
                                    nc.vector.tensor_tensor(t2[:, :, :], SHt[:, :, 0:2],
                                                            SHt[:, :, 2:4], OP.add)
                                    nc.vector.tensor_tensor(sc[:, :, :], sc[:, :, :], t2[:, :, :],
                                                            OP.add)
                                    nc.vector.scalar_tensor_tensor(
                                        sc[:, :, :], sc[:, :, :], LRELU, sc[:, :, :],
                                        OP.mult, OP.max)
                                    nc.scalar.activation(At[:, slot0:slot0 + PC, :],
                                                         sc[:, :, :], AF.Exp)

                                # messages: Z table is (d, h)-interleaved so At rides
                                # as a packed innermost-2 operand (2x DVE mode)
                                if it == 0:
                                    # it0 rows are plain [h(64)|s_t|res|1|..]: write
                                    # interleaved msg[(d,h)] = h[d]*At[h]
                                    msg = msgp.tile([128, PC, 128], BF16, tag="msg")
                                    mv = msg.rearrange("p c (d o) -> p c d o", o=2)
                                    for o in range(2):
                                        nc.vector.tensor_tensor(
                                            mv[:, :, :, o],
                                            Gt[:, :, 0:64],
                                            At[:, slot0:slot0 + PC, o].unsqueeze(2)
                                                .broadcast_to([128, PC, 64]),
                                            OP.mult)
                                else:
                                    msg = msgp.tile([128, PC, 128], BF16, tag="msg")
                                    nc.vector.tensor_tensor(
                                        msg.rearrange("p c (d o) -> p c d o", o=2),
                                        Gt.rearrange("p c (d o) -> p c d o", o=2),
                                        At[:, slot0:slot0 + PC, :].unsqueeze(2)
                                            .broadcast_to([128, PC, 64, 2]),
                                        OP.mult)
                                # spmm (+ z columns via separate 2-col matmul on iter 0)
                                # PSUM zero-regions are whole banks: start/stop once per bank.
                                for j in range(PC):
                                    bl = BLKMAP[j]
                                    st = (p == 0) and (j == 0 or j == J_B1)
                                    sp = (p == 1) and (j == J_B1 - 1 or j == PC - 1)
                                    nc.tensor.matmul(
                                        psb[:, bl, :], oh[:, :, j], msg[:, j, :],
                                        start=st, stop=sp)
                                    if it == 0:
                                        nc.tensor.matmul(
                                            psz[:, bl, :], oh[:, :, j],
                                            At[:, slot0 + j, :],
                                            start=(p == 0 and j == 0),
                                            stop=(p == 1 and j == PC - 1))

                            # piece epilogue: z, recip, Z assembly
                            if it == 0:
                                b0 = k * BPP
                                nc.vector.tensor_scalar(zt[:, b0:b0 + BPP, :], psz[:, :, :],
                                                        1e-30, None, OP.max)
                                nc.vector.reciprocal(recip[:, b0:b0 + BPP, :],
                                                     zt[:, b0:b0 + BPP, :])
                                nc.vector.tensor_scalar(recip[:, b0:b0 + BPP, :],
                                                        recip[:, b0:b0 + BPP, :],
                                                        1.0 - ALPHA, None, OP.mult)
                            zv = Z.rearrange("p b (d o) -> p b d o", o=2)
                            pv = psb.rearrange("p b (d o) -> p b d o", o=2)
                            for bl in range(BPP):
                                b = k * BPP + bl
                                nc.scalar.activation(zv[:, b, :, 0], pv[:, bl, :, 0], AF.Copy,
                                                     scale=recip[:, b, 0:1])
                                nc.scalar.activation(zv[:, b, :, 1], pv[:, bl, :, 1],
                                                     AF.Copy, scale=recip[:, b, 1:2])
                                nc.vector.scalar_tensor_tensor(
                                    zv[:, b, :, :],
                                    h_t[:, b, :].unsqueeze(2)
                                        .broadcast_to([128, 64, 2]),
                                    ALPHA,
                                    zv[:, b, :, :],
                                    OP.mult, OP.add)
                            if it < n_iters - 1:
                                # stream this piece's table rows out now so the
                                # next iteration's first gather isn't serialized
                                # behind a full-table write
                                tzp = ep2.tile([128, BPP, 128], BF16, tag="tzp")
                                nc.vector.tensor_copy(tzp[:, :, :],
                                                      Z[:, k * BPP:(k + 1) * BPP, :])
                                tdst = tab_in if do_ag else table
                                r0 = k * BPP * 128
                                nc.sync.dma_start(
                                    tdst[r0:r0 + BPP * 128, :]
                                        .rearrange("(b p) c -> p b c", p=128),
                                    tzp[:, :, :])

                        if debug and l == 0 and it == 0:
                            nc.sync.dma_start(dbg_z0.ap().rearrange("(b p) c -> p b c", p=128),
                                              Z[:, :, :])
                            nc.sync.dma_start(
                                dbg_at.ap(),
                                At[:, :, :])

                        if it < n_iters - 1 and do_ag:
                            nc.gpsimd.collective_compute(
                                "AllGather", OP.bypass,
                                replica_groups=[list(range(CORES))],
                                ins=[tab_in.opt()], outs=[table.opt()])

                # ================= conv + residual =================
                with tc.tile_pool(name="convp", bufs=2) as cp, \
                     tc.tile_pool(name="convps", bufs=4, space="PSUM") as cps:
                    for b in range(NBLK):
                        pzt = cps.tile([128, 128], F32, tag="pzt")
                        nc.tensor.transpose(pzt[:, :], Z[:, b, :], idf[:, :])
                        Zt = cp.tile([128, 128], F32, tag="Zt")
                        nc.scalar.activation(Zt[:, :], pzt[:, :], AF.Copy)
                        pc_ = cps.tile([128, 64], F32, tag="pc")
                        nc.tensor.matmul(pc_[:, :], Zt[:, :], Wo[:, l, :],
                                         start=True, stop=True)
                        nc.vector.tensor_tensor(ent[:, b, :], pc_[:, :], ent[:, b, :], OP.add)

            nc.sync.dma_start(out_ext.ap().rearrange("(b p) f -> p b f", p=128),
                              ent[:, :, :])

    nc.compile()
    return nc


# ----------------------------------------------------------------------------
# host wrapper
# ----------------------------------------------------------------------------

def _make_in_maps(inputs, plans):
    ent = np.asarray(inputs["entity_embed"], np.float32)
    rel = np.zeros((512, DIM), np.float32)
    rel[:N_REL] = np.asarray(inputs["relation_embed"], np.float32)
    lng = np.tile(np.asarray(inputs["ln_gamma"], np.float32)[None], (128, 1, 1))
    lnb = np.tile(np.asarray(inputs["ln_beta"], np.float32)[None], (128, 1, 1))
    W = np.stack([np.asarray(inputs["W_h"], np.float32),
                  np.asarray(inputs["W_t"], np.float32),
                  np.asarray(inputs["W_r"], np.float32)], axis=0)
    att = np.stack([np.asarray(inputs["att_h"], np.float32),
                    np.asarray(inputs["att_t"], np.float32),
                    np.asarray(inputs["att_r"], np.float32)], axis=0)
    att_rep = np.tile(att.reshape(3, L_LAYERS, 1, HD), (1, 1, 128, 1)).astype(np.float32)
    Wo = np.asarray(inputs["W_o"], np.float32)
    # Z columns are (d, h)-interleaved on device; permute W_o rows to match
    il = np.arange(HD)
    Wo = np.ascontiguousarray(Wo[:, (il % 2) * DIM + il // 2, :])
    iota = np.tile(np.arange(128, dtype=np.float32)[:, None], (1, PC))
    iota = np.tile(iota[None], (128, 1, 1)).astype(BF)
    idf = np.eye(128, dtype=np.float32)

    common = dict(rel_emb=rel, ln_g=np.ascontiguousarray(lng), ln_b=np.ascontiguousarray(lnb),
                  W_htr=np.ascontiguousarray(W), att_rep=np.ascontiguousarray(att_rep),
                  W_o=Wo, iota_exp=np.ascontiguousarray(iota), ident_f=idf)
    in_maps = []
    for pl in plans:
        sl = np.zeros((M_PER_CORE, DIM), np.float32)
        nv = pl["n_valid"]
        sl[pl["perm"][:nv]] = ent[pl["node_lo"]:pl["node_lo"] + nv]
        m = dict(common)
        m["ent_slice"] = sl
        m["gidx"] = np.ascontiguousarray(np.stack(pl["gidx"], axis=1))
        m["tidx"] = pl["tidx"]
        m["shidx"] = pl["shidx"]
        m["offs"] = pl["offs"]
        in_maps.append(m)
    return in_maps


_CACHE = {}


def _get_nc(meta_key, meta, debug):
    key = (meta_key, debug)
    if key not in _CACHE:
        _CACHE[key] = _build_nc(meta, debug=debug)
    return _CACHE[key]


def run(inputs, debug=False, trace=False):
    plans, meta = _build_plan(np.asarray(inputs["edge_index"]),
                              np.asarray(inputs["edge_type"]))
    nc = _get_nc((meta["C_PASS"],), meta, debug)
    in_maps = _make_in_maps(inputs, plans)
    res = bass_utils.run_bass_kernel_spmd(nc, in_maps, core_ids=list(range(CORES)),
                                          trace=trace)
    out = np.zeros((N_ENT, DIM), np.float32)
    for c, pl in enumerate(plans):
        nv = pl["n_valid"]
        sl = np.asarray(res.results[c]["out_slice"])
        out[pl["node_lo"]:pl["node_lo"] + nv] = sl[pl["perm"][:nv]]
    return out, res, plans, meta


def kernel(**inputs) -> np.ndarray:
    out, _, _, _ = run(inputs)
    return out.astype(np.asarray(inputs["entity_embed"]).dtype)
